# revision 1
# baseline (speedup 1.0000x reference)
"""Trainium2 Bass kernel for nn_CLinear_6768868459230.

Context-conditioned block-autoregressive linear layer (MAF-style):
  wdir = c @ Wd + bd                      [B, O, I]
  w    = exp(wdir)*mask_diag + wdir*mask_lower
  sqn  = sum(w^2, axis=I)
  y    = (w / sqrt(sqn) * exp(wamp)) @ xv + bias
  logdet = logsumexp over diag block of (wdir - 0.5 log sqn + wamp + xl)

Sharding: tensor-parallel over the O=512 output rows (the 262144-wide Wd
matmul dominates). Each of the 8 cores owns 8 of the 64 channels, chosen
as pairs {k, 15-k, 16+k, 31-k, ...} so the block-triangular work (rows of
channel ch touch only ch*8 input columns) is identical on every core —
required anyway because all cores execute one shared program.

Only the strictly-lower + diagonal columns of Wd are shipped/loaded
(the rest are masked to zero by the reference), roughly halving traffic.
Per-row lower widths are zero-padded up to a per-slot maximum W(j)=64j+56
so the instruction stream is core-independent; zero padding is exact
(contributes 0 to both sums).

On-device per core (per 128-sample batch chunk):
  TensorE : wdir lower segments + diag block via cT.T @ Wd (float32r),
            with bd added as K=1 ones-outer-product accumulating matmuls
  ScalarE : per-row sum(t^2) via activation(Square, accum_out)
  VectorE : per-row sum(t * xv) via tensor_tensor_reduce
  diag    : batched 512-wide: exp(td), exp(2 td), products with xv and
            exp(xl), segmented tensor_reduce over fin=8
  logdet  = wamp - 0.5*ln(sqn) + ln(sum_f exp(td + xl))  (no max-trick
            needed: |td + xl| <~ 8 at these scales)
"""

import numpy as np

NCH, FIN, FOUT, CDIM, B = 64, 8, 8, 128, 256
I = NCH * FIN
O = NCH * FOUT
NCORES = 8
NLOC = 64  # output rows per core
BCH = 128  # batch chunk (SBUF partitions)

# per-slot padded lower width and rows-per-matmul grouping
W_OF_J = [64 * j + 56 for j in range(8)]
G_OF_J = [8, 4, 2, 2, 1, 1, 1, 1]  # rows per matmul so N = G*W <= 512
BASE_J = [8 * sum(W_OF_J[:j]) for j in range(8)]
TOTW = 8 * sum(W_OF_J)  # 17920


def _channels(k):
    return [k, 15 - k, 16 + k, 31 - k, 32 + k, 47 - k, 48 + k, 63 - k]


_NC = None
_REPEAT = 1  # bench knob: replicate compute body


def _build_nc():
    import concourse.bacc as bacc
    import concourse.tile as tile
    from concourse import mybir

    f32 = mybir.dt.float32
    f32r = mybir.dt.float32r
    bf16 = mybir.dt.bfloat16
    AF = mybir.ActivationFunctionType
    ALU = mybir.AluOpType

    nc = bacc.Bacc(None, target_bir_lowering=False)

    d_cT = nc.dram_tensor("ct", [CDIM, B], bf16, kind="ExternalInput")
    d_cTf = nc.dram_tensor("ctf", [CDIM, B], f32r, kind="ExternalInput")
    d_wdl = nc.dram_tensor("wdl", [CDIM, TOTW], bf16, kind="ExternalInput")
    d_wdd = nc.dram_tensor("wdd", [CDIM, NLOC * FIN], f32r, kind="ExternalInput")
    d_bdd = nc.dram_tensor("bdd", [1, NLOC * FIN], f32r, kind="ExternalInput")
    d_ew = nc.dram_tensor("ew", [CDIM, 3 * NLOC], bf16, kind="ExternalInput")
    d_eb = nc.dram_tensor("eb", [1, 3 * NLOC], bf16, kind="ExternalInput")
    d_xvT = nc.dram_tensor("xvt", [I, B], bf16, kind="ExternalInput")
    d_bdm = nc.dram_tensor("bdm", [I, NLOC], bf16, kind="ExternalInput")
    d_xv = nc.dram_tensor("xv", [B, I], f32, kind="ExternalInput")
    d_xvd = nc.dram_tensor("xvd", [B, NLOC * FIN], f32, kind="ExternalInput")
    d_xle = nc.dram_tensor("xle", [B, NLOC * FIN], f32, kind="ExternalInput")
    d_out = nc.dram_tensor("out", [B, NLOC, 2], f32, kind="ExternalOutput")

    with tile.TileContext(nc) as tc:
        with (
            tc.tile_pool(name="consts", bufs=1) as consts,
            tc.tile_pool(name="scr", bufs=3) as scr,
            tc.tile_pool(name="accs", bufs=2) as accs,
            tc.tile_pool(name="segp", bufs=6, space="PSUM") as segp,
            tc.tile_pool(name="miscp", bufs=1, space="PSUM") as miscp,
            tc.tile_pool(name="extp", bufs=1, space="PSUM") as extp,
        ):
            # ---- constants / weights ----
            # sync queue: ct + xv first (gate the matmuls / dot products),
            # then even weight slots. scalar queue: small matmul operands,
            # then odd weight slots. The two HW DGEs stream in parallel.
            ct_sb = consts.tile([CDIM, B], bf16)
            nc.sync.dma_start(out=ct_sb, in_=d_cT[:, :])
            ctf_sb = consts.tile([CDIM, B], f32r)
            nc.sync.dma_start(out=ctf_sb, in_=d_cTf[:, :])
            onesf_sb = consts.tile([1, BCH], f32r)
            nc.vector.memset(onesf_sb.bitcast(mybir.dt.uint32), 0x3F800000)
            ones_sb = consts.tile([1, BCH], bf16)
            nc.vector.memset(ones_sb, 1.0)
            xv_sb, xvd_sb, xle_sb = [], [], []
            for bc in range(2):
                b0 = bc * BCH
                t = consts.tile([BCH, I], f32, name=f"xv{bc}", tag=f"xv{bc}")
                nc.sync.dma_start(out=t, in_=d_xv[b0 : b0 + BCH, :])
                xv_sb.append(t)
            wdd_sb = consts.tile([CDIM, NLOC * FIN], f32r)
            nc.scalar.dma_start(out=wdd_sb, in_=d_wdd[:, :])
            bdd_sb = consts.tile([1, NLOC * FIN], f32r)
            nc.scalar.dma_start(out=bdd_sb, in_=d_bdd[:, :])
            ew_sb = consts.tile([CDIM, 3 * NLOC], bf16)
            nc.scalar.dma_start(out=ew_sb, in_=d_ew[:, :])
            eb_sb = consts.tile([1, 3 * NLOC], bf16)
            nc.scalar.dma_start(out=eb_sb, in_=d_eb[:, :])
            xvt_sb = consts.tile([CDIM, 4, B], bf16)
            for kc in range(4):
                nc.scalar.dma_start(
                    out=xvt_sb[:, kc, :], in_=d_xvT[kc * 128 : (kc + 1) * 128, :]
                )
            bdm_sb = consts.tile([CDIM, 4, NLOC], bf16)
            for kc in range(4):
                nc.scalar.dma_start(
                    out=bdm_sb[:, kc, :], in_=d_bdm[kc * 128 : (kc + 1) * 128, :]
                )
            wdl_sb = [None] * 8
            ENG_OF_J = ["sync", "scalar", "sync", "scalar",
                        "sync", "gps", "scalar", "gps"]
            for j in (0, 1, 2, 3, 4, 5, 6, 7):
                w = W_OF_J[j]
                t = consts.tile([CDIM, 8 * w], bf16, name=f"wdl{j}", tag=f"wdl{j}")
                eng = {"sync": nc.sync, "scalar": nc.scalar, "gps": nc.gpsimd}[
                    ENG_OF_J[j]
                ]
                eng.dma_start(out=t, in_=d_wdl[:, BASE_J[j] : BASE_J[j] + 8 * w])
                wdl_sb[j] = t
            for bc in range(2):
                b0 = bc * BCH
                t = consts.tile([BCH, NLOC * FIN], f32, name=f"xvd{bc}", tag=f"xvd{bc}")
                nc.sync.dma_start(out=t, in_=d_xvd[b0 : b0 + BCH, :])
                xvd_sb.append(t)
                t = consts.tile([BCH, NLOC * FIN], f32, name=f"xle{bc}", tag=f"xle{bc}")
                nc.sync.dma_start(out=t, in_=d_xle[b0 : b0 + BCH, :])
                xle_sb.append(t)

            for _rep in range(_REPEAT):
                BATCH_SQ = (0, 1)  # slots whose squares are segment-batched
                st = {}
                # ---- phase A: matmuls + squares + dot products ----
                for bc in range(2):
                    b0 = bc * BCH
                    lhs = ct_sb[:, b0 : b0 + BCH]
                    xv_b = xv_sb[bc]

                    # extras: wamp | bias | 2*c@g | dotbd
                    pex = extp.tile([BCH, 4 * NLOC], f32, name="pex", tag="pex")
                    nc.tensor.matmul(
                        pex[:, : 3 * NLOC], lhs, ew_sb, start=True, stop=False
                    )
                    nc.tensor.matmul(
                        pex[:, : 3 * NLOC], ones_sb, eb_sb, start=False, stop=True
                    )
                    for kc in range(4):
                        nc.tensor.matmul(
                            pex[:, 3 * NLOC :],
                            xvt_sb[:, kc, b0 : b0 + BCH],
                            bdm_sb[:, kc, :],
                            start=(kc == 0),
                            stop=(kc == 3),
                        )

                    # diag block matmul (elementwise work deferred to phase B)
                    pdg = miscp.tile([BCH, NLOC * FIN], f32, name="pdg", tag="pdg")
                    nc.tensor.matmul(pdg, ctf_sb[:, b0 : b0 + BCH], wdd_sb, start=True, stop=False)
                    nc.tensor.matmul(pdg, onesf_sb, bdd_sb, start=False, stop=True)

                    SQL = accs.tile([BCH, NLOC], f32, name="SQL", tag="SQL")
                    DOTL = accs.tile([BCH, NLOC], f32, name="DOTL", tag="DOTL")
                    sq_pend = []
                    for j in range(8):
                        w, g = W_OF_J[j], G_OF_J[j]
                        prodj = scr.tile(
                            [BCH, 8 * 504], f32, name="prodj", tag="prodj", bufs=2
                        )
                        sqbj = None
                        if j in BATCH_SQ:
                            sqbj = scr.tile(
                                [BCH, 8 * 120], f32, name="sqbj", tag="sqbj", bufs=2
                            )
                        for s in range(8 // g):
                            r0 = j * 8 + s * g
                            n = g * w
                            ps = segp.tile([BCH, 512], f32, name="ps", tag="ps")
                            nc.tensor.matmul(
                                ps[:, :n],
                                lhs,
                                wdl_sb[j][:, s * n : (s + 1) * n],
                                start=True,
                                stop=True,
                            )
                            if j in BATCH_SQ:
                                nc.scalar.activation(
                                    out=sqbj[:, s * n : (s + 1) * n],
                                    in_=ps[:, :n],
                                    func=AF.Square,
                                )
                            else:
                                for q in range(g):
                                    r = r0 + q
                                    a = q * w
                                    sS = scr.tile(
                                        [BCH, 504], f32, name="sS", tag="sS"
                                    )
                                    nc.scalar.activation(
                                        out=sS[:, :w],
                                        in_=ps[:, a : a + w],
                                        func=AF.Square,
                                        accum_out=SQL[:, r : r + 1],
                                    )
                            # t * xv for all g rows (xv broadcast over rows)
                            if g == 1:
                                nc.vector.tensor_mul(
                                    prodj[:, s * n : (s + 1) * n],
                                    ps[:, :n],
                                    xv_b[:, :w],
                                )
                            else:
                                nc.vector.tensor_mul(
                                    prodj[:, s * n : (s + 1) * n].rearrange(
                                        "p (g w) -> p g w", w=w
                                    ),
                                    ps[:, :n].rearrange("p (g w) -> p g w", w=w),
                                    xv_b[:, :w].unsqueeze(1).broadcast_to(
                                        [BCH, g, w]
                                    ),
                                )
                        nc.vector.tensor_reduce(
                            out=DOTL[:, j * 8 : (j + 1) * 8],
                            in_=prodj[:, : 8 * w].rearrange("p (r w) -> p r w", w=w),
                            axis=mybir.AxisListType.X,
                            op=ALU.add,
                        )
                        if j in BATCH_SQ:
                            sq_pend.append((j, w, sqbj))
                    for j, w, sqbj in sq_pend:
                        nc.vector.tensor_reduce(
                            out=SQL[:, j * 8 : (j + 1) * 8],
                            in_=sqbj[:, : 8 * w].rearrange("p (r w) -> p r w", w=w),
                            axis=mybir.AxisListType.X,
                            op=ALU.add,
                        )
                    st[bc] = dict(pex=pex, pdg=pdg, SQL=SQL, DOTL=DOTL)

                # ---- phase B: diag elementwise (all Exp — one table set) ----
                for bc in range(2):
                    s_ = st[bc]
                    pdg = s_["pdg"]
                    expd = scr.tile(
                        [BCH, NLOC * FIN], f32, name="expd", tag="expd", bufs=2
                    )
                    nc.scalar.activation(out=expd, in_=pdg, func=AF.Exp)
                    sq2 = scr.tile(
                        [BCH, NLOC * FIN], f32, name="sq2", tag="sq2", bufs=2
                    )
                    nc.scalar.activation(out=sq2, in_=pdg, func=AF.Exp, scale=2.0)
                    SQD = accs.tile([BCH, NLOC], f32, name="SQD", tag="SQD")
                    nc.vector.tensor_reduce(
                        out=SQD,
                        in_=sq2.rearrange("p (r f) -> p r f", f=FIN),
                        axis=mybir.AxisListType.X,
                        op=ALU.add,
                    )
                    prd = scr.tile(
                        [BCH, NLOC * FIN], f32, name="prd", tag="prd", bufs=2
                    )
                    nc.gpsimd.tensor_mul(prd, expd, xvd_sb[bc])
                    DOTD = accs.tile([BCH, NLOC], f32, name="DOTD", tag="DOTD")
                    nc.vector.tensor_reduce(
                        out=DOTD,
                        in_=prd.rearrange("p (r f) -> p r f", f=FIN),
                        axis=mybir.AxisListType.X,
                        op=ALU.add,
                    )
                    prl = scr.tile(
                        [BCH, NLOC * FIN], f32, name="prl", tag="prl", bufs=2
                    )
                    nc.gpsimd.tensor_mul(prl, expd, xle_sb[bc])
                    LDS = accs.tile([BCH, NLOC], f32, name="LDS", tag="LDS")
                    nc.vector.tensor_reduce(
                        out=LDS,
                        in_=prl.rearrange("p (r f) -> p r f", f=FIN),
                        axis=mybir.AxisListType.X,
                        op=ALU.add,
                    )
                    s_.update(SQD=SQD, DOTD=DOTD, LDS=LDS)

                # ---- phase C: assembly. ScalarE order: Ln*4, Copy*2, Exp*2 ----
                for bc in range(2):
                    s_ = st[bc]
                    sqn = accs.tile([BCH, NLOC], f32, name="sqn", tag="sqn")
                    nc.vector.tensor_add(sqn, s_["SQL"], s_["SQD"])
                    nc.vector.tensor_add(
                        sqn, sqn, s_["pex"][:, 2 * NLOC : 3 * NLOC]
                    )
                    dot = accs.tile([BCH, NLOC], f32, name="dot", tag="dot")
                    nc.vector.tensor_add(dot, s_["DOTL"], s_["DOTD"])
                    nc.vector.tensor_add(dot, dot, s_["pex"][:, 3 * NLOC :])
                    s_.update(sqn=sqn, dot=dot)
                for bc in range(2):
                    s_ = st[bc]
                    l1 = accs.tile([BCH, NLOC], f32, name="l1", tag="l1")
                    nc.scalar.activation(out=l1, in_=s_["sqn"], func=AF.Ln)
                    l2 = accs.tile([BCH, NLOC], f32, name="l2", tag="l2")
                    nc.scalar.activation(out=l2, in_=s_["LDS"], func=AF.Ln)
                    s_.update(l1=l1, l2=l2)
                for bc in range(2):
                    s_ = st[bc]
                    m1 = accs.tile([BCH, NLOC], f32, name="m1", tag="m1")
                    nc.scalar.mul(m1, s_["l1"], -0.5)
                    u = accs.tile([BCH, NLOC], f32, name="u", tag="u")
                    nc.vector.tensor_add(u, s_["pex"][:, :NLOC], m1)
                    s_.update(u=u)
                for bc in range(2):
                    s_ = st[bc]
                    sc = accs.tile([BCH, NLOC], f32, name="sc", tag="sc")
                    nc.scalar.activation(out=sc, in_=s_["u"], func=AF.Exp)
                    s_.update(sc=sc)
                for bc in range(2):
                    b0 = bc * BCH
                    s_ = st[bc]
                    yv = accs.tile([BCH, NLOC], f32, name="yv", tag="yv")
                    nc.vector.tensor_mul(yv, s_["dot"], s_["sc"])
                    yb = accs.tile([BCH, NLOC], f32, name="yb", tag="yb")
                    nc.vector.tensor_add(yb, yv, s_["pex"][:, NLOC : 2 * NLOC])
                    ld = accs.tile([BCH, NLOC], f32, name="ld", tag="ld")
                    nc.vector.tensor_add(ld, s_["u"], s_["l2"])
                    ob = accs.tile([BCH, NLOC, 2], f32, name="ob", tag="ob")
                    nc.gpsimd.tensor_copy(out=ob[:, :, 0], in_=yb)
                    nc.gpsimd.tensor_copy(out=ob[:, :, 1], in_=ld)
                    nc.sync.dma_start(out=d_out[b0 : b0 + BCH, :, :], in_=ob)

    nc.compile()
    return nc


def _host_prep(x, c, Wd, bd, Wa, ba, Wb, bb):
    """Build the 8 per-core input maps."""
    import ml_dtypes

    bf = ml_dtypes.bfloat16
    x = np.ascontiguousarray(x, dtype=np.float32)
    c = np.ascontiguousarray(c, dtype=np.float32)
    Wd5 = np.ascontiguousarray(Wd, dtype=np.float32).reshape(CDIM, NCH, FOUT, NCH, FIN)
    bd4 = np.ascontiguousarray(bd, dtype=np.float32).reshape(NCH, FOUT, NCH, FIN)
    Wa = np.ascontiguousarray(Wa, dtype=np.float32)
    Wb = np.ascontiguousarray(Wb, dtype=np.float32)
    ba = np.ascontiguousarray(ba, dtype=np.float32)
    bb = np.ascontiguousarray(bb, dtype=np.float32)

    cT = np.ascontiguousarray(c.T)
    xv = np.ascontiguousarray(x[:, :, 0])
    xl = np.ascontiguousarray(x[:, :, 1])
    xvT = np.ascontiguousarray(xv.T)

    in_maps = []
    for k in range(NCORES):
        chs = _channels(k)
        wdl = np.zeros((CDIM, TOTW), dtype=np.float32)
        wdd = np.empty((CDIM, NLOC * FIN), dtype=np.float32)
        bdd = np.empty((1, NLOC * FIN), dtype=np.float32)
        ew = np.zeros((CDIM, 3 * NLOC), dtype=np.float32)
        eb = np.zeros((1, 3 * NLOC), dtype=np.float32)
        bdm = np.zeros((I, NLOC), dtype=np.float32)
        xvd = np.empty((B, NLOC * FIN), dtype=np.float32)
        xled = np.empty((B, NLOC * FIN), dtype=np.float32)
        for j, ch in enumerate(chs):
            w = W_OF_J[j]
            for fo in range(FOUT):
                lo = BASE_J[j] + fo * w
                wdl[:, lo : lo + ch * FIN] = Wd5[:, ch, fo, :ch, :].reshape(CDIM, -1)
                r = j * FOUT + fo
                wdd[:, r * FIN : (r + 1) * FIN] = Wd5[:, ch, fo, ch, :]
                bdd[0, r * FIN : (r + 1) * FIN] = bd4[ch, fo, ch, :]
                bd_low = bd4[ch, fo, :ch, :].reshape(-1)  # true lower bd row
                bdm[: ch * FIN, r] = bd_low
                # sqn cross term 2*(c@g) and constant sum(bd^2)
                wl = Wd5[:, ch, fo, :ch, :].reshape(CDIM, -1)
                ew[:, 2 * NLOC + r] = 2.0 * (wl @ bd_low)
                eb[0, 2 * NLOC + r] = np.dot(bd_low, bd_low)
            rows = slice(ch * FOUT, (ch + 1) * FOUT)
            ew[:, j * FOUT : (j + 1) * FOUT] = Wa[:, rows]
            ew[:, NLOC + j * FOUT : NLOC + (j + 1) * FOUT] = Wb[:, rows]
            eb[0, j * FOUT : (j + 1) * FOUT] = ba[rows]
            eb[0, NLOC + j * FOUT : NLOC + (j + 1) * FOUT] = bb[rows]
        for r in range(NLOC):
            j = r // FOUT
            ch = chs[j]
            xvd[:, r * FIN : (r + 1) * FIN] = xv[:, ch * FIN : (ch + 1) * FIN]
            xled[:, r * FIN : (r + 1) * FIN] = xl[:, ch * FIN : (ch + 1) * FIN]
        np.exp(xled, out=xled)

        in_maps.append(
            {
                "ct": cT.astype(bf),
                "ctf": cT,
                "wdl": wdl.astype(bf),
                "wdd": wdd,
                "bdd": bdd,
                "ew": ew.astype(bf),
                "eb": eb.astype(bf),
                "xv": xv,
                "xvt": xvT.astype(bf),
                "bdm": bdm.astype(bf),
                "xvd": xvd,
                "xle": xled,
            }
        )
    return in_maps


def kernel(x, c, Wd, bd, Wa, ba, Wb, bb, _trace=False, _tmpdir=None):
    global _NC
    from concourse.bass_utils import run_bass_kernel_spmd

    if _NC is None:
        _NC = _build_nc()
    in_maps = _host_prep(x, c, Wd, bd, Wa, ba, Wb, bb)
    res = run_bass_kernel_spmd(
        _NC, in_maps, core_ids=list(range(NCORES)), trace=_trace, tmpdir=_tmpdir
    )

    out = np.empty((B, O, 2), dtype=np.float32)
    for k in range(NCORES):
        ok = res.results[k]["out"]
        for j, ch in enumerate(_channels(k)):
            out[:, ch * FOUT : (ch + 1) * FOUT, :] = ok[:, j * FOUT : (j + 1) * FOUT, :]
    if _trace:
        return out, res
    return out



# revision 16
# speedup vs baseline: 1.4678x; 1.4678x over previous
"""Trainium2 Bass kernel for nn_CLinear_6768868459230 (V-formulation).

Context-conditioned block-autoregressive linear layer (MAF-style):
  wdir = c @ Wd + bd                      [B, O, I]
  w    = exp(wdir)*mask_diag + wdir*mask_lower
  sqn  = sum(w^2, axis=I)
  y    = (w / sqrt(sqn) * exp(wamp)) @ xv + bias
  logdet = logsumexp over diag block of (wdir - 0.5 log sqn + wamp + xl)

Sharding: tensor-parallel over the O=512 output rows. Each of the 8 cores
owns 8 of the 64 channels, chosen as {k, 15-k, 16+k, 31-k, ...} so the
per-slot lower-width window count is identical on every core (shared
program).

Per row r the only heavy quantities are the two lower-block reductions
  B_r[b] = sum_i t[b,r,i]*xv[b,i]   and   A_r[b] = sum_i t[b,r,i]^2
with t = c @ W_r. Neither requires materializing t in batch-major layout:

  B_r = c . V_r           V_r = W_r @ xv   (PE, contract over i in
                          128-wide windows, accumulate in PSUM; [128k, 256b])
        then one DVE multiply V_r * cT and a ones-column selector matmul
        on the PE reduces over the 128 k-partitions.
  A_r = ||L_r^T c||^2     L_r from a host-side Cholesky of the Gram
                          W_r W_r^T (compresses rows wider than 128), plus
                          host-folded cross/const terms for bd. Squares on
                          ScalarE, partition-reduce again via PE selector.

All reductions therefore run on the PE at 128-way parallelism instead of
DVE tensor_reduce (which has no fast modes), the batch is processed as one
256-wide free dim, and the only ScalarE work is one squares pass + a few
exps/lns. Weights ship as bf16; all accumulation is f32 in PSUM.
"""

import numpy as np

NCH, FIN, FOUT, CDIM, B = 64, 8, 8, 128, 256
I = NCH * FIN
O = NCH * FOUT
NCORES = 8
NLOC = 64  # output rows per core
WIN_OF_J = [1, 1, 2, 2, 3, 3, 4, 4]  # 128-wide i-windows per slot
NBLK = 8 * sum(WIN_OF_J)  # 160 weight blocks per core


def _channels(k):
    return [k, 15 - k, 16 + k, 31 - k, 32 + k, 47 - k, 48 + k, 63 - k]


_NC = None
_DEBUG = False


def _build_nc():
    import concourse.bacc as bacc
    import concourse.tile as tile
    from concourse import mybir
    from concourse.masks import make_identity

    f32 = mybir.dt.float32
    bf16 = mybir.dt.bfloat16
    AF = mybir.ActivationFunctionType

    nc = bacc.Bacc(None, target_bir_lowering=False)

    d_ct = nc.dram_tensor("ct", [CDIM, B], bf16, kind="ExternalInput")
    d_wtk = nc.dram_tensor("wtk", [128, NBLK * 128], bf16, kind="ExternalInput")
    d_lch = nc.dram_tensor("lch", [128, NLOC * 128], bf16, kind="ExternalInput")
    d_xvt = nc.dram_tensor("xvt", [128, 4 * B], bf16, kind="ExternalInput")
    d_wdd = nc.dram_tensor("wdd", [128, 512], bf16, kind="ExternalInput")
    d_bdd = nc.dram_tensor("bdd", [1, 512], bf16, kind="ExternalInput")
    d_xvdt = nc.dram_tensor("xvdt", [128, 4 * B], bf16, kind="ExternalInput")
    d_exlt = nc.dram_tensor("exlt", [128, 4 * B], bf16, kind="ExternalInput")
    d_sel16 = nc.dram_tensor("sel16", [128, 4 * 64], bf16, kind="ExternalInput")
    d_sel64 = nc.dram_tensor("sel64", [128, 64 * 64], bf16, kind="ExternalInput")
    d_wampq = nc.dram_tensor("wampq", [128, 64], bf16, kind="ExternalInput")
    d_biasq = nc.dram_tensor("biasq", [128, 64], bf16, kind="ExternalInput")
    d_eba = nc.dram_tensor("eba", [1, 64], bf16, kind="ExternalInput")
    d_ebb = nc.dram_tensor("ebb", [1, 64], bf16, kind="ExternalInput")
    d_crossq = nc.dram_tensor("crossq", [128, 64], bf16, kind="ExternalInput")
    d_ebc = nc.dram_tensor("ebc", [1, 64], bf16, kind="ExternalInput")
    d_bdm = nc.dram_tensor("bdm", [128, 4 * 64], bf16, kind="ExternalInput")
    d_out = nc.dram_tensor("out", [B, NLOC, 2], f32, kind="ExternalOutput")
    if _DEBUG:
        d_dbg_dot = nc.dram_tensor("dbg_dot", [64, 256], f32, kind="ExternalOutput")
        d_dbg_v = nc.dram_tensor("dbg_v", [128, 2, 256], f32, kind="ExternalOutput")
        d_dbg_p = nc.dram_tensor("dbg_p", [128, 2, 256], f32, kind="ExternalOutput")

    # wtk block ranges per slot j (for DMA splitting)
    blk_start = [8 * sum(WIN_OF_J[:j]) for j in range(8)]

    with tile.TileContext(nc) as tc:
        with (
            tc.tile_pool(name="consts", bufs=1) as consts,
            tc.tile_pool(name="scrP", bufs=6) as scrP,
            tc.tile_pool(name="scrZ", bufs=6) as scrZ,
            tc.tile_pool(name="scrD", bufs=2) as scrD,
            tc.tile_pool(name="asm", bufs=1) as asm,
            tc.tile_pool(name="pVp", bufs=2, space="PSUM") as pVp,
            tc.tile_pool(name="pZp", bufs=2, space="PSUM") as pZp,
            tc.tile_pool(name="paccp", bufs=1, space="PSUM") as paccp,
            tc.tile_pool(name="ptdp", bufs=1, space="PSUM") as ptdp,
            tc.tile_pool(name="pexp", bufs=1, space="PSUM") as pexp,
        ):
            # ---- DMAs ----
            ct_sb = consts.tile([CDIM, B], bf16)
            nc.sync.dma_start(out=ct_sb, in_=d_ct[:, :])
            xvt_sb = consts.tile([128, 4, B], bf16)
            nc.sync.dma_start(out=xvt_sb, in_=d_xvt[:, :])
            wdd_sb = consts.tile([128, 512], bf16)
            nc.sync.dma_start(out=wdd_sb, in_=d_wdd[:, :])
            bdd_sb = consts.tile([1, 512], bf16)
            nc.sync.dma_start(out=bdd_sb, in_=d_bdd[:, :])
            lch_sb = consts.tile([128, NLOC, 128], bf16)
            for g in range(4):
                nc.sync.dma_start(
                    out=lch_sb[:, 16 * g : 16 * (g + 1), :],
                    in_=d_lch[:, 16 * g * 128 : 16 * (g + 1) * 128],
                )
            wampq_sb = consts.tile([128, 64], bf16)
            nc.sync.dma_start(out=wampq_sb, in_=d_wampq[:, :])
            biasq_sb = consts.tile([128, 64], bf16)
            nc.sync.dma_start(out=biasq_sb, in_=d_biasq[:, :])
            eba_sb = consts.tile([1, 64], bf16)
            nc.sync.dma_start(out=eba_sb, in_=d_eba[:, :])
            ebb_sb = consts.tile([1, 64], bf16)
            nc.sync.dma_start(out=ebb_sb, in_=d_ebb[:, :])
            crossq_sb = consts.tile([128, 64], bf16)
            nc.sync.dma_start(out=crossq_sb, in_=d_crossq[:, :])
            ebc_sb = consts.tile([1, 64], bf16)
            nc.sync.dma_start(out=ebc_sb, in_=d_ebc[:, :])
            bdm_sb = consts.tile([128, 4, 64], bf16)
            nc.sync.dma_start(out=bdm_sb, in_=d_bdm[:, :])
            sel16_sb = consts.tile([128, 4, 64], bf16)
            nc.sync.dma_start(out=sel16_sb, in_=d_sel16[:, :])
            sel64_sb = consts.tile([128, 64, 64], bf16)
            nc.sync.dma_start(out=sel64_sb, in_=d_sel64[:, :])

            wtk_sb = consts.tile([128, NBLK, 128], bf16)
            for j in range(6):  # scalar queue: slots 0-5
                b0, b1 = blk_start[j], blk_start[j] + 8 * WIN_OF_J[j]
                nc.scalar.dma_start(
                    out=wtk_sb[:, b0:b1, :], in_=d_wtk[:, b0 * 128 : b1 * 128]
                )
            for j in (6, 7):  # gpsimd queue: slots 6-7
                b0, b1 = blk_start[j], blk_start[j] + 8 * WIN_OF_J[j]
                nc.gpsimd.dma_start(
                    out=wtk_sb[:, b0:b1, :], in_=d_wtk[:, b0 * 128 : b1 * 128]
                )
            xvdt_sb = consts.tile([128, 4, B], bf16)
            nc.gpsimd.dma_start(out=xvdt_sb, in_=d_xvdt[:, :])
            exlt_sb = consts.tile([128, 4, B], bf16)
            nc.gpsimd.dma_start(out=exlt_sb, in_=d_exlt[:, :])

            ones1 = consts.tile([1, B], bf16)
            nc.vector.memset(ones1, 1.0)
            id64 = consts.tile([64, 64], f32)
            make_identity(nc, id64)

            ct_b2 = ct_sb.unsqueeze(1).broadcast_to([CDIM, 2, B])

            # ---- acc regions: 0=sqn, 1=dot, 2=LDS, 3=wamp ----
            acc = paccp.tile([64, 4, 256], f32, name="acc", tag="acc")

            # extras: wamp -> acc[:,3]; bias -> pex then SBUF
            nc.tensor.matmul(
                acc[:, 3, :], wampq_sb, ct_sb, start=True, stop=False,
                skip_group_check=True,
            )
            nc.tensor.matmul(
                acc[:, 3, :], eba_sb, ones1, start=False, stop=False,
                skip_group_check=True,
            )
            pex = pexp.tile([128, 256], f32, name="pex", tag="pex")
            nc.tensor.matmul(
                pex[0:64, :], biasq_sb, ct_sb, start=True, stop=False,
                skip_group_check=True,
            )
            nc.tensor.matmul(
                pex[0:64, :], ebb_sb, ones1, start=False, stop=True,
                skip_group_check=True,
            )
            bias_sb = asm.tile([64, 256], f32, name="bias", tag="bias")
            nc.scalar.copy(bias_sb, pex[0:64, :])

            # diag produce + exps (early PE warmup; sels come later)
            E_g, E2_g = [], []
            for g in range(2):
                ptd = ptdp.tile([128, 2, 256], f32, name="ptd", tag="ptd")
                for u in range(2):
                    a = 2 * g + u
                    nc.tensor.matmul(
                        ptd[:, u, :],
                        wdd_sb[:, a * 128 : (a + 1) * 128],
                        ct_sb,
                        start=(u == 0),
                        stop=False,
                        skip_group_check=True,
                    )
                    nc.tensor.matmul(
                        ptd[:, u, :],
                        bdd_sb[:, a * 128 : (a + 1) * 128],
                        ones1,
                        start=False,
                        stop=(u == 1),
                        skip_group_check=True,
                    )
                E = scrD.tile([128, 2, 256], bf16, name="E", tag="E")
                nc.scalar.activation(out=E, in_=ptd, func=AF.Exp)
                E2 = scrD.tile([128, 2, 256], bf16, name="E2", tag="E2")
                nc.scalar.activation(out=E2, in_=ptd, func=AF.Exp, scale=2.0)
                E_g.append(E)
                E2_g.append(E2)

            # ---- V pass: produce V row-pairs, DVE product with cT, B-sels ----
            blk = 0
            pend = []
            for t in range(32):
                pV = pVp.tile([128, 2, 256], f32, name="pV", tag="pV")
                for q in range(2):
                    r = 2 * t + q
                    nw = WIN_OF_J[r // 8]
                    for a in range(nw):
                        nc.tensor.matmul(
                            pV[:, q, :],
                            wtk_sb[:, blk, :],
                            xvt_sb[:, a, :],
                            start=(q == 0 and a == 0),
                            stop=(q == 1 and a == nw - 1),
                            skip_group_check=True,
                        )
                        blk += 1
                P = scrP.tile([128, 2, 256], bf16, name="P", tag="P")
                nc.vector.tensor_mul(P, pV, ct_b2)
                if _DEBUG and t == 0:
                    dbgv = asm.tile([128, 2, 256], f32, name="dbgv", tag="dbgv")
                    nc.vector.tensor_copy(out=dbgv, in_=pV)
                    nc.sync.dma_start(out=d_dbg_v[:, :, :], in_=dbgv)
                    dbgp = asm.tile([128, 2, 256], f32, name="dbgp", tag="dbgp")
                    nc.gpsimd.tensor_copy(out=dbgp, in_=P)
                    nc.sync.dma_start(out=d_dbg_p[:, :, :], in_=dbgp)
                pend.append((t, P))
                if len(pend) > 2:
                    tq, Pq = pend.pop(0)
                    for q in range(2):
                        r = 2 * tq + q
                        nc.tensor.matmul(
                            acc[:, 1, :],
                            sel64_sb[:, r, :],
                            Pq[:, q, :],
                            start=(r == 0),
                            stop=False,
                            skip_group_check=True,
                        )
            for tq, Pq in pend:
                for q in range(2):
                    r = 2 * tq + q
                    nc.tensor.matmul(
                        acc[:, 1, :],
                        sel64_sb[:, r, :],
                        Pq[:, q, :],
                        start=(r == 0),
                        stop=False,
                        skip_group_check=True,
                    )
            pend = []

            # ---- z pass: z = L^T c, squares on ScalarE, z-sels ----
            for t in range(32):
                pz = pZp.tile([128, 2, 256], f32, name="pz", tag="pz")
                for q in range(2):
                    r = 2 * t + q
                    nc.tensor.matmul(
                        pz[:, q, :], lch_sb[:, r, :], ct_sb,
                        start=(q == 0), stop=(q == 1), skip_group_check=True,
                    )
                zq = scrZ.tile([128, 2, 256], bf16, name="zsq", tag="zsq")
                nc.scalar.activation(out=zq, in_=pz, func=AF.Square)
                pend.append((t, zq))
                if len(pend) > 2:
                    tq, Zq = pend.pop(0)
                    for q in range(2):
                        r = 2 * tq + q
                        nc.tensor.matmul(
                            acc[:, 0, :],
                            sel64_sb[:, r, :],
                            Zq[:, q, :],
                            start=False,
                            stop=False,
                            skip_group_check=True,
                        )
            for tq, Zq in pend:
                for q in range(2):
                    r = 2 * tq + q
                    nc.tensor.matmul(
                        acc[:, 0, :],
                        sel64_sb[:, r, :],
                        Zq[:, q, :],
                        start=False,
                        stop=False,
                        skip_group_check=True,
                    )

            # ---- late accumulators into sqn / dot ----
            nc.tensor.matmul(
                acc[:, 0, :], crossq_sb, ct_sb, start=False, stop=False,
                skip_group_check=True,
            )
            nc.tensor.matmul(
                acc[:, 0, :], ebc_sb, ones1, start=False, stop=False,
                skip_group_check=True,
            )
            for a in range(4):
                nc.tensor.matmul(
                    acc[:, 1, :],
                    bdm_sb[:, a, :],
                    xvt_sb[:, a, :],
                    start=False,
                    stop=False,
                    skip_group_check=True,
                )
            # diag products (xvdt/exlt arrive late; DVE is free here)
            Pd_g, Pl_g = [], []
            for g in range(2):
                Pd = scrD.tile([128, 2, 256], bf16, name="Pd", tag="Pd")
                nc.vector.tensor_mul(Pd, E_g[g], xvdt_sb[:, 2 * g : 2 * g + 2, :])
                Pl = scrD.tile([128, 2, 256], bf16, name="Pl", tag="Pl")
                nc.vector.tensor_mul(Pl, E_g[g], exlt_sb[:, 2 * g : 2 * g + 2, :])
                Pd_g.append(Pd)
                Pl_g.append(Pl)
            for g in range(2):
                for u in range(2):
                    a = 2 * g + u
                    nc.tensor.matmul(
                        acc[:, 0, :], sel16_sb[:, a, :], E2_g[g][:, u, :],
                        start=False, stop=False, skip_group_check=True,
                    )
                    nc.tensor.matmul(
                        acc[:, 1, :], sel16_sb[:, a, :], Pd_g[g][:, u, :],
                        start=False, stop=(a == 3), skip_group_check=True,
                    )
                    nc.tensor.matmul(
                        acc[:, 2, :], sel16_sb[:, a, :], Pl_g[g][:, u, :],
                        start=False, stop=(a == 3), skip_group_check=True,
                    )

            if _DEBUG:
                dbgd = asm.tile([64, 256], f32, name="dbgd", tag="dbgd")
                nc.vector.tensor_copy(out=dbgd, in_=acc[:, 1, :])
                nc.sync.dma_start(out=d_dbg_dot[:, :], in_=dbgd)

            # ---- assembly (all [64, 256]) ----
            l1 = asm.tile([64, 256], f32, name="l1", tag="l1")
            nc.scalar.activation(out=l1, in_=acc[:, 0, :], func=AF.Ln)
            l2 = asm.tile([64, 256], f32, name="l2", tag="l2")
            nc.scalar.activation(out=l2, in_=acc[:, 2, :], func=AF.Ln)
            mh = asm.tile([64, 256], f32, name="mh", tag="mh")
            nc.scalar.mul(mh, l1, -0.5)
            u_t = asm.tile([64, 256], f32, name="u", tag="u")
            nc.vector.tensor_add(u_t, acc[:, 3, :], mh)
            sc = asm.tile([64, 256], f32, name="sc", tag="sc")
            nc.scalar.activation(out=sc, in_=u_t, func=AF.Exp)
            yv = asm.tile([64, 256], f32, name="yv", tag="yv")
            nc.vector.tensor_mul(yv, acc[:, 1, :], sc)
            yb = asm.tile([64, 256], f32, name="yb", tag="yb")
            nc.vector.tensor_add(yb, yv, bias_sb)
            ld = asm.tile([64, 256], f32, name="ld", tag="ld")
            nc.vector.tensor_add(ld, u_t, l2)

            # ---- transpose to [B, NLOC] and write out ----
            pT = pexp.tile([128, 256], f32, name="pex", tag="pex")
            for i, (src, o0) in enumerate(
                ((yb, 0), (yb, 64), (ld, 128), (ld, 192))
            ):
                nc.tensor.matmul(
                    pT[:, o0 : o0 + 64],
                    src[:, (i % 2) * 128 : (i % 2) * 128 + 128],
                    id64,
                    is_transpose=True,
                    start=(i == 0),
                    stop=(i == 3),
                    skip_group_check=True,
                )
            for h in range(2):
                ob = asm.tile([128, 64, 2], f32, name=f"ob{h}", tag=f"ob{h}")
                nc.vector.tensor_copy(out=ob[:, :, 0], in_=pT[:, 64 * h : 64 * h + 64])
                nc.vector.tensor_copy(
                    out=ob[:, :, 1], in_=pT[:, 128 + 64 * h : 192 + 64 * h]
                )
                nc.sync.dma_start(out=d_out[128 * h : 128 * (h + 1), :, :], in_=ob)

    nc.compile()
    return nc


def _host_prep(x, c, Wd, bd, Wa, ba, Wb, bb):
    """Build the 8 per-core input maps."""
    import ml_dtypes

    bf = ml_dtypes.bfloat16
    x = np.ascontiguousarray(x, dtype=np.float32)
    c = np.ascontiguousarray(c, dtype=np.float32)
    Wd5 = np.ascontiguousarray(Wd, dtype=np.float32).reshape(CDIM, NCH, FOUT, NCH, FIN)
    bd4 = np.ascontiguousarray(bd, dtype=np.float32).reshape(NCH, FOUT, NCH, FIN)
    Wa = np.asarray(Wa, dtype=np.float32)
    Wb = np.asarray(Wb, dtype=np.float32)
    ba = np.asarray(ba, dtype=np.float32)
    bb = np.asarray(bb, dtype=np.float32)

    cT = np.ascontiguousarray(c.T)  # [128, 256]
    xv = x[:, :, 0]
    xl = x[:, :, 1]
    xvT = np.ascontiguousarray(xv.T)  # [512, 256]
    exlT = np.exp(xl).T  # [512, 256]

    def fold4(a512):  # [512, N] -> [128, 4*N] window-major per partition
        N = a512.shape[1]
        return np.ascontiguousarray(
            a512.reshape(4, 128, N).transpose(1, 0, 2).reshape(128, 4 * N)
        )

    xvt = fold4(xvT).astype(bf)

    sel16 = np.zeros((128, 4, 64), dtype=np.float32)
    p = np.arange(128)
    for a in range(4):
        sel16[p, a, 16 * a + p // 8] = 1.0
    sel16 = np.ascontiguousarray(sel16.reshape(128, 256)).astype(bf)
    sel64 = np.ascontiguousarray(
        np.broadcast_to(np.eye(64, dtype=np.float32), (128, 64, 64)).reshape(128, 4096)
    ).astype(bf)

    in_maps = []
    for k in range(NCORES):
        chs = _channels(k)
        blocks = []
        lch = np.zeros((128, NLOC, 128), dtype=np.float32)
        crossq = np.zeros((128, 64), dtype=np.float32)
        ebc = np.zeros((1, 64), dtype=np.float32)
        bdm = np.zeros((512, 64), dtype=np.float32)
        wdd = np.empty((128, 512), dtype=np.float32)
        bdd = np.empty((1, 512), dtype=np.float32)
        xvd_cols = np.empty((512, B), dtype=np.float32)
        exl_cols = np.empty((512, B), dtype=np.float32)
        wampq = np.empty((128, 64), dtype=np.float32)
        biasq = np.empty((128, 64), dtype=np.float32)
        eba = np.empty((1, 64), dtype=np.float32)
        ebb = np.empty((1, 64), dtype=np.float32)

        for j, ch in enumerate(chs):
            w = 8 * ch
            nw = WIN_OF_J[j]
            arr = Wd5[:, ch, :, :ch, :].reshape(CDIM, FOUT, w)  # [k, q, w]
            arrp = np.zeros((CDIM, FOUT, 128 * nw), dtype=np.float32)
            arrp[:, :, :w] = arr
            # blocks in (q, a) order: [q, a, i, k]
            bl = arrp.reshape(CDIM, FOUT, nw, 128).transpose(1, 2, 3, 0)
            blocks.append(np.ascontiguousarray(bl.reshape(FOUT * nw, 128, CDIM)))
            bdj = bd4[ch, :, :ch, :].reshape(FOUT, w)  # [q, w]
            if w >= 128:
                a64 = arr.astype(np.float64)
                G = np.matmul(a64.transpose(1, 0, 2), a64.transpose(1, 2, 0))
                tr = np.trace(G, axis1=1, axis2=2)
                G += np.eye(CDIM)[None] * (1e-9 * tr[:, None, None] / CDIM)
                L = np.linalg.cholesky(G)  # [q, 128, 128], G = L @ L.T
                for q in range(FOUT):
                    lch[:, j * 8 + q, :] = L[q]
            else:
                for q in range(FOUT):
                    lch[:, j * 8 + q, :w] = arr[:, q, :]
            for q in range(FOUT):
                r = j * 8 + q
                crossq[:, r] = 2.0 * (arr[:, q, :] @ bdj[q])
                ebc[0, r] = bdj[q] @ bdj[q]
                bdm[:w, r] = bdj[q]
                wdd[:, r * 8 : (r + 1) * 8] = Wd5[:, ch, q, ch, :]
                bdd[0, r * 8 : (r + 1) * 8] = bd4[ch, q, ch, :]
                xvd_cols[r * 8 : (r + 1) * 8, :] = xvT[8 * ch : 8 * ch + 8, :]
                exl_cols[r * 8 : (r + 1) * 8, :] = exlT[8 * ch : 8 * ch + 8, :]
            rows = slice(ch * FOUT, (ch + 1) * FOUT)
            wampq[:, j * 8 : (j + 1) * 8] = Wa[:, rows]
            biasq[:, j * 8 : (j + 1) * 8] = Wb[:, rows]
            eba[0, j * 8 : (j + 1) * 8] = ba[rows]
            ebb[0, j * 8 : (j + 1) * 8] = bb[rows]

        wtk = np.concatenate(blocks, axis=0)  # [160, 128, 128] (blk, i, k)
        wtk = np.ascontiguousarray(
            wtk.transpose(1, 0, 2).reshape(128, NBLK * 128)
        )

        in_maps.append(
            {
                "ct": cT.astype(bf),
                "wtk": wtk.astype(bf),
                "lch": np.ascontiguousarray(lch.reshape(128, NLOC * 128)).astype(bf),
                "xvt": xvt,
                "wdd": wdd.astype(bf),
                "bdd": bdd.astype(bf),
                "xvdt": fold4(xvd_cols).astype(bf),
                "exlt": fold4(exl_cols).astype(bf),
                "sel16": sel16,
                "sel64": sel64,
                "wampq": wampq.astype(bf),
                "biasq": biasq.astype(bf),
                "eba": eba.astype(bf),
                "ebb": ebb.astype(bf),
                "crossq": crossq.astype(bf),
                "ebc": ebc.astype(bf),
                "bdm": fold4(bdm).astype(bf),
            }
        )
    return in_maps


def kernel(x, c, Wd, bd, Wa, ba, Wb, bb, _trace=False, _tmpdir=None):
    global _NC
    from concourse.bass_utils import run_bass_kernel_spmd

    if _NC is None:
        _NC = _build_nc()
    in_maps = _host_prep(x, c, Wd, bd, Wa, ba, Wb, bb)
    res = run_bass_kernel_spmd(
        _NC, in_maps, core_ids=list(range(NCORES)), trace=_trace, tmpdir=_tmpdir
    )

    out = np.empty((B, O, 2), dtype=np.float32)
    for k in range(NCORES):
        ok = res.results[k]["out"]
        for j, ch in enumerate(_channels(k)):
            out[:, ch * FOUT : (ch + 1) * FOUT, :] = ok[:, j * FOUT : (j + 1) * FOUT, :]
    if _trace:
        return out, res
    return out


# revision 17
# speedup vs baseline: 1.6155x; 1.1006x over previous
"""Trainium2 Bass kernel for nn_CLinear_6768868459230 (V-formulation).

Context-conditioned block-autoregressive linear layer (MAF-style):
  wdir = c @ Wd + bd                      [B, O, I]
  w    = exp(wdir)*mask_diag + wdir*mask_lower
  sqn  = sum(w^2, axis=I)
  y    = (w / sqrt(sqn) * exp(wamp)) @ xv + bias
  logdet = logsumexp over diag block of (wdir - 0.5 log sqn + wamp + xl)

Sharding: tensor-parallel over the O=512 output rows. Each of the 8 cores
owns 8 of the 64 channels, chosen as {k, 15-k, 16+k, 31-k, ...} so the
per-slot lower-width window count is identical on every core (shared
program).

Per row r the only heavy quantities are the two lower-block reductions
  B_r[b] = sum_i t[b,r,i]*xv[b,i]   and   A_r[b] = sum_i t[b,r,i]^2
with t = c @ W_r. Neither requires materializing t in batch-major layout:

  B_r = c . V_r           V_r = W_r @ xv   (PE, contract over i in
                          128-wide windows, accumulate in PSUM; [128k, 256b])
        then one DVE multiply V_r * cT and a ones-column selector matmul
        on the PE reduces over the 128 k-partitions.
  A_r = ||L_r^T c||^2     L_r from a host-side Cholesky of the Gram
                          W_r W_r^T (compresses rows wider than 128), plus
                          host-folded cross/const terms for bd. Squares on
                          ScalarE, partition-reduce again via PE selector.

All reductions therefore run on the PE at 128-way parallelism instead of
DVE tensor_reduce (which has no fast modes), the batch is processed as one
256-wide free dim, and the only ScalarE work is one squares pass + a few
exps/lns. Weights ship as bf16; all accumulation is f32 in PSUM.
"""

import numpy as np

NCH, FIN, FOUT, CDIM, B = 64, 8, 8, 128, 256
I = NCH * FIN
O = NCH * FOUT
NCORES = 8
NLOC = 64  # output rows per core
WIN_OF_J = [1, 1, 2, 2, 3, 3, 4, 4]  # 128-wide i-windows per slot
NBLK = 8 * sum(WIN_OF_J)  # 160 weight blocks per core


def _channels(k):
    return [k, 15 - k, 16 + k, 31 - k, 32 + k, 47 - k, 48 + k, 63 - k]


_NC = None
_DEBUG = False


def _build_nc():
    import concourse.bacc as bacc
    import concourse.tile as tile
    from concourse import mybir
    from concourse.masks import make_identity

    f32 = mybir.dt.float32
    bf16 = mybir.dt.bfloat16
    AF = mybir.ActivationFunctionType

    nc = bacc.Bacc(None, target_bir_lowering=False)

    d_ct = nc.dram_tensor("ct", [CDIM, B], bf16, kind="ExternalInput")
    d_wtk = nc.dram_tensor("wtk", [128, NBLK * 128], bf16, kind="ExternalInput")
    d_lch = nc.dram_tensor("lch", [128, NLOC * 128], bf16, kind="ExternalInput")
    d_xvt = nc.dram_tensor("xvt", [128, 4 * B], bf16, kind="ExternalInput")
    d_wdd = nc.dram_tensor("wdd", [128, 512], bf16, kind="ExternalInput")
    d_bdd = nc.dram_tensor("bdd", [1, 512], bf16, kind="ExternalInput")
    d_xvdt = nc.dram_tensor("xvdt", [128, 4 * B], bf16, kind="ExternalInput")
    d_exlt = nc.dram_tensor("exlt", [128, 4 * B], bf16, kind="ExternalInput")
    d_sel16 = nc.dram_tensor("sel16", [128, 4 * 64], bf16, kind="ExternalInput")
    d_sel64 = nc.dram_tensor("sel64", [128, 64 * 64], bf16, kind="ExternalInput")
    d_wampq = nc.dram_tensor("wampq", [128, 64], bf16, kind="ExternalInput")
    d_biasq = nc.dram_tensor("biasq", [128, 64], bf16, kind="ExternalInput")
    d_eba = nc.dram_tensor("eba", [1, 64], bf16, kind="ExternalInput")
    d_ebb = nc.dram_tensor("ebb", [1, 64], bf16, kind="ExternalInput")
    d_crossq = nc.dram_tensor("crossq", [128, 64], bf16, kind="ExternalInput")
    d_ebc = nc.dram_tensor("ebc", [1, 64], bf16, kind="ExternalInput")
    d_bdm = nc.dram_tensor("bdm", [128, 4 * 64], bf16, kind="ExternalInput")
    d_out = nc.dram_tensor("out", [B, NLOC, 2], f32, kind="ExternalOutput")
    if _DEBUG:
        d_dbg_dot = nc.dram_tensor("dbg_dot", [64, 256], f32, kind="ExternalOutput")
        d_dbg_v = nc.dram_tensor("dbg_v", [128, 2, 256], f32, kind="ExternalOutput")
        d_dbg_p = nc.dram_tensor("dbg_p", [128, 2, 256], f32, kind="ExternalOutput")

    # wtk block ranges per slot j (for DMA splitting)
    blk_start = [8 * sum(WIN_OF_J[:j]) for j in range(8)]

    with tile.TileContext(nc) as tc:
        with (
            tc.tile_pool(name="consts", bufs=1) as consts,
            tc.tile_pool(name="scrP", bufs=6) as scrP,
            tc.tile_pool(name="scrZ", bufs=6) as scrZ,
            tc.tile_pool(name="scrD", bufs=2) as scrD,
            tc.tile_pool(name="asm", bufs=1) as asm,
            tc.tile_pool(name="pVp", bufs=2, space="PSUM") as pVp,
            tc.tile_pool(name="pZp", bufs=2, space="PSUM") as pZp,
            tc.tile_pool(name="paccp", bufs=1, space="PSUM") as paccp,
            tc.tile_pool(name="ptdp", bufs=1, space="PSUM") as ptdp,
            tc.tile_pool(name="pexp", bufs=1, space="PSUM") as pexp,
        ):
            # ---- DMAs ----
            # sync queue: small gating tensors first, then lch; wtk big
            # streams ride the scalar + gpsimd queues.
            ct_sb = consts.tile([CDIM, B], bf16)
            nc.sync.dma_start(out=ct_sb, in_=d_ct[:, :])
            xvt_sb = consts.tile([128, 4, B], bf16)
            nc.sync.dma_start(out=xvt_sb, in_=d_xvt[:, :])
            wdd_sb = consts.tile([128, 512], bf16)
            nc.sync.dma_start(out=wdd_sb, in_=d_wdd[:, :])
            bdd_sb = consts.tile([1, 512], bf16)
            nc.sync.dma_start(out=bdd_sb, in_=d_bdd[:, :])
            wampq_sb = consts.tile([128, 64], bf16)
            nc.sync.dma_start(out=wampq_sb, in_=d_wampq[:, :])
            biasq_sb = consts.tile([128, 64], bf16)
            nc.sync.dma_start(out=biasq_sb, in_=d_biasq[:, :])
            eba_sb = consts.tile([1, 64], bf16)
            nc.sync.dma_start(out=eba_sb, in_=d_eba[:, :])
            ebb_sb = consts.tile([1, 64], bf16)
            nc.sync.dma_start(out=ebb_sb, in_=d_ebb[:, :])
            crossq_sb = consts.tile([128, 64], bf16)
            nc.sync.dma_start(out=crossq_sb, in_=d_crossq[:, :])
            ebc_sb = consts.tile([1, 64], bf16)
            nc.sync.dma_start(out=ebc_sb, in_=d_ebc[:, :])
            bdm_sb = consts.tile([128, 4, 64], bf16)
            nc.sync.dma_start(out=bdm_sb, in_=d_bdm[:, :])
            sel16_sb = consts.tile([128, 4, 64], bf16)
            nc.sync.dma_start(out=sel16_sb, in_=d_sel16[:, :])
            xvdt_sb = consts.tile([128, 4, B], bf16)
            nc.sync.dma_start(out=xvdt_sb, in_=d_xvdt[:, :])
            exlt_sb = consts.tile([128, 4, B], bf16)
            nc.sync.dma_start(out=exlt_sb, in_=d_exlt[:, :])
            sel64_sb = consts.tile([128, 64, 64], bf16)
            nc.sync.dma_start(out=sel64_sb, in_=d_sel64[:, :])
            lch_sb = consts.tile([128, NLOC, 128], bf16)
            for g in range(4):
                nc.sync.dma_start(
                    out=lch_sb[:, 16 * g : 16 * (g + 1), :],
                    in_=d_lch[:, 16 * g * 128 : 16 * (g + 1) * 128],
                )

            wtk_sb = consts.tile([128, NBLK, 128], bf16)
            for j in range(6):  # scalar queue: slots 0-5
                b0, b1 = blk_start[j], blk_start[j] + 8 * WIN_OF_J[j]
                nc.scalar.dma_start(
                    out=wtk_sb[:, b0:b1, :], in_=d_wtk[:, b0 * 128 : b1 * 128]
                )
            for j in (6, 7):  # gpsimd queue: slots 6-7
                b0, b1 = blk_start[j], blk_start[j] + 8 * WIN_OF_J[j]
                nc.gpsimd.dma_start(
                    out=wtk_sb[:, b0:b1, :], in_=d_wtk[:, b0 * 128 : b1 * 128]
                )

            ones1 = consts.tile([1, B], bf16)
            nc.vector.memset(ones1, 1.0)
            id64 = consts.tile([64, 64], f32)
            make_identity(nc, id64)

            ct_b2 = ct_sb.unsqueeze(1).broadcast_to([CDIM, 2, B])

            # ---- acc regions: 0=sqn, 1=dot, 2=LDS, 3=wamp ----
            acc = paccp.tile([64, 4, 256], f32, name="acc", tag="acc")

            # extras: wamp -> acc[:,3]; bias -> pex then SBUF
            nc.tensor.matmul(
                acc[:, 3, :], wampq_sb, ct_sb, start=True, stop=False,
                skip_group_check=True,
            )
            nc.tensor.matmul(
                acc[:, 3, :], eba_sb, ones1, start=False, stop=False,
                skip_group_check=True,
            )
            pex = pexp.tile([128, 256], f32, name="pex", tag="pex")
            nc.tensor.matmul(
                pex[0:64, :], biasq_sb, ct_sb, start=True, stop=False,
                skip_group_check=True,
            )
            nc.tensor.matmul(
                pex[0:64, :], ebb_sb, ones1, start=False, stop=True,
                skip_group_check=True,
            )
            bias_sb = asm.tile([64, 256], f32, name="bias", tag="bias")
            nc.scalar.copy(bias_sb, pex[0:64, :])

            # diag produce + exps (early PE warmup; sels come later)
            E_g, E2_g = [], []
            for g in range(2):
                ptd = ptdp.tile([128, 2, 256], f32, name="ptd", tag="ptd")
                for u in range(2):
                    a = 2 * g + u
                    nc.tensor.matmul(
                        ptd[:, u, :],
                        wdd_sb[:, a * 128 : (a + 1) * 128],
                        ct_sb,
                        start=(u == 0),
                        stop=False,
                        skip_group_check=True,
                    )
                    nc.tensor.matmul(
                        ptd[:, u, :],
                        bdd_sb[:, a * 128 : (a + 1) * 128],
                        ones1,
                        start=False,
                        stop=(u == 1),
                        skip_group_check=True,
                    )
                E = scrD.tile([128, 2, 256], bf16, name="E", tag="E")
                nc.scalar.activation(out=E, in_=ptd, func=AF.Exp)
                E2 = scrD.tile([128, 2, 256], bf16, name="E2", tag="E2")
                nc.scalar.activation(out=E2, in_=ptd, func=AF.Exp, scale=2.0)
                E_g.append(E)
                E2_g.append(E2)

            # ---- merged pair loop: V + z produce, DVE/ScalarE fill the
            # comb tile (zsq | P), one 512-wide selector matmul per row.
            # Late accumulators (cross/dotbd/diag sels) are sprinkled into
            # the loop so the post-loop tail is minimal.
            late = []
            late.append(lambda: nc.tensor.matmul(
                acc[:, 0, :], crossq_sb, ct_sb, start=False, stop=False,
                skip_group_check=True))
            late.append(lambda: nc.tensor.matmul(
                acc[:, 0, :], ebc_sb, ones1, start=False, stop=False,
                skip_group_check=True))
            for a_ in range(4):
                late.append(lambda a=a_: nc.tensor.matmul(
                    acc[:, 1, :], bdm_sb[:, a, :], xvt_sb[:, a, :],
                    start=False, stop=False, skip_group_check=True))
            # diag products for dot/LDS (DVE) are emitted up front; their
            # selector matmuls ride the late list.
            Pd_g, Pl_g = [], []
            for g in range(2):
                Pd = scrD.tile([128, 2, 256], bf16, name="Pd", tag="Pd")
                nc.vector.tensor_mul(Pd, E_g[g], xvdt_sb[:, 2 * g : 2 * g + 2, :])
                Pl = scrD.tile([128, 2, 256], bf16, name="Pl", tag="Pl")
                nc.vector.tensor_mul(Pl, E_g[g], exlt_sb[:, 2 * g : 2 * g + 2, :])
                Pd_g.append(Pd)
                Pl_g.append(Pl)
            for g in range(2):
                for u in range(2):
                    a_ = 2 * g + u
                    late.append(lambda g=g, u=u, a=a_: nc.tensor.matmul(
                        acc[:, 0, :], sel16_sb[:, a, :], E2_g[g][:, u, :],
                        start=False, stop=False, skip_group_check=True))
                    late.append(lambda g=g, u=u, a=a_: nc.tensor.matmul(
                        acc[:, 1, :], sel16_sb[:, a, :], Pd_g[g][:, u, :],
                        start=False, stop=False, skip_group_check=True))
                    late.append(lambda g=g, u=u, a=a_: nc.tensor.matmul(
                        acc[:, 2, :], sel16_sb[:, a, :], Pl_g[g][:, u, :],
                        start=False, stop=(a == 3), skip_group_check=True))

            blk = 0
            pend = []
            for t in range(32):
                pV = pVp.tile([128, 2, 256], f32, name="pV", tag="pV")
                for q in range(2):
                    r = 2 * t + q
                    nw = WIN_OF_J[r // 8]
                    for a in range(nw):
                        nc.tensor.matmul(
                            pV[:, q, :],
                            wtk_sb[:, blk, :],
                            xvt_sb[:, a, :],
                            start=(q == 0 and a == 0),
                            stop=(q == 1 and a == nw - 1),
                            skip_group_check=True,
                        )
                        blk += 1
                pz = pZp.tile([128, 2, 256], f32, name="pz", tag="pz")
                for q in range(2):
                    r = 2 * t + q
                    nc.tensor.matmul(
                        pz[:, q, :], lch_sb[:, r, :], ct_sb,
                        start=(q == 0), stop=(q == 1), skip_group_check=True,
                    )
                comb = scrP.tile([128, 2, 2, B], bf16, name="comb", tag="comb")
                nc.vector.tensor_mul(comb[:, :, 1, :], pV, ct_b2)
                nc.scalar.activation(out=comb[:, :, 0, :], in_=pz, func=AF.Square)
                pend.append((t, comb))
                if len(pend) > 3:
                    tq, cq = pend.pop(0)
                    for q in range(2):
                        r = 2 * tq + q
                        nc.tensor.matmul(
                            acc[:, 0:2, :],
                            sel64_sb[:, r, :],
                            cq[:, q, :, :],
                            start=(r == 0),
                            stop=False,
                            skip_group_check=True,
                        )
                    if t >= 8 and late:
                        late.pop(0)()
                        if late:
                            late.pop(0)()
            for tq, cq in pend:
                for q in range(2):
                    r = 2 * tq + q
                    nc.tensor.matmul(
                        acc[:, 0:2, :],
                        sel64_sb[:, r, :],
                        cq[:, q, :, :],
                        start=(r == 0),
                        stop=(r == 63 and not late),
                        skip_group_check=True,
                    )
            for fn in late:
                fn()
            late = []

            if _DEBUG:
                dbgd = asm.tile([64, 256], f32, name="dbgd", tag="dbgd")
                nc.vector.tensor_copy(out=dbgd, in_=acc[:, 1, :])
                nc.sync.dma_start(out=d_dbg_dot[:, :], in_=dbgd)

            # ---- assembly (all [64, 256]) ----
            l1 = asm.tile([64, 256], f32, name="l1", tag="l1")
            nc.scalar.activation(out=l1, in_=acc[:, 0, :], func=AF.Ln)
            l2 = asm.tile([64, 256], f32, name="l2", tag="l2")
            nc.scalar.activation(out=l2, in_=acc[:, 2, :], func=AF.Ln)
            mh = asm.tile([64, 256], f32, name="mh", tag="mh")
            nc.scalar.mul(mh, l1, -0.5)
            u_t = asm.tile([64, 256], f32, name="u", tag="u")
            nc.vector.tensor_add(u_t, acc[:, 3, :], mh)
            sc = asm.tile([64, 256], f32, name="sc", tag="sc")
            nc.scalar.activation(out=sc, in_=u_t, func=AF.Exp)
            yv = asm.tile([64, 256], f32, name="yv", tag="yv")
            nc.vector.tensor_mul(yv, acc[:, 1, :], sc)
            yb = asm.tile([64, 256], f32, name="yb", tag="yb")
            nc.vector.tensor_add(yb, yv, bias_sb)
            ld = asm.tile([64, 256], f32, name="ld", tag="ld")
            nc.vector.tensor_add(ld, u_t, l2)

            # ---- transpose to [B, NLOC] and write out ----
            pT = pexp.tile([128, 256], f32, name="pex", tag="pex")
            for i, (src, o0) in enumerate(
                ((yb, 0), (yb, 64), (ld, 128), (ld, 192))
            ):
                nc.tensor.matmul(
                    pT[:, o0 : o0 + 64],
                    src[:, (i % 2) * 128 : (i % 2) * 128 + 128],
                    id64,
                    is_transpose=True,
                    start=(i == 0),
                    stop=(i == 3),
                    skip_group_check=True,
                )
            for h in range(2):
                ob = asm.tile([128, 64, 2], f32, name=f"ob{h}", tag=f"ob{h}")
                nc.vector.tensor_copy(out=ob[:, :, 0], in_=pT[:, 64 * h : 64 * h + 64])
                nc.vector.tensor_copy(
                    out=ob[:, :, 1], in_=pT[:, 128 + 64 * h : 192 + 64 * h]
                )
                nc.sync.dma_start(out=d_out[128 * h : 128 * (h + 1), :, :], in_=ob)

    nc.compile()
    return nc


def _host_prep(x, c, Wd, bd, Wa, ba, Wb, bb):
    """Build the 8 per-core input maps."""
    import ml_dtypes

    bf = ml_dtypes.bfloat16
    x = np.ascontiguousarray(x, dtype=np.float32)
    c = np.ascontiguousarray(c, dtype=np.float32)
    Wd5 = np.ascontiguousarray(Wd, dtype=np.float32).reshape(CDIM, NCH, FOUT, NCH, FIN)
    bd4 = np.ascontiguousarray(bd, dtype=np.float32).reshape(NCH, FOUT, NCH, FIN)
    Wa = np.asarray(Wa, dtype=np.float32)
    Wb = np.asarray(Wb, dtype=np.float32)
    ba = np.asarray(ba, dtype=np.float32)
    bb = np.asarray(bb, dtype=np.float32)

    cT = np.ascontiguousarray(c.T)  # [128, 256]
    xv = x[:, :, 0]
    xl = x[:, :, 1]
    xvT = np.ascontiguousarray(xv.T)  # [512, 256]
    exlT = np.exp(xl).T  # [512, 256]

    def fold4(a512):  # [512, N] -> [128, 4*N] window-major per partition
        N = a512.shape[1]
        return np.ascontiguousarray(
            a512.reshape(4, 128, N).transpose(1, 0, 2).reshape(128, 4 * N)
        )

    xvt = fold4(xvT).astype(bf)

    sel16 = np.zeros((128, 4, 64), dtype=np.float32)
    p = np.arange(128)
    for a in range(4):
        sel16[p, a, 16 * a + p // 8] = 1.0
    sel16 = np.ascontiguousarray(sel16.reshape(128, 256)).astype(bf)
    sel64 = np.ascontiguousarray(
        np.broadcast_to(np.eye(64, dtype=np.float32), (128, 64, 64)).reshape(128, 4096)
    ).astype(bf)

    in_maps = []
    for k in range(NCORES):
        chs = _channels(k)
        blocks = []
        lch = np.zeros((128, NLOC, 128), dtype=np.float32)
        crossq = np.zeros((128, 64), dtype=np.float32)
        ebc = np.zeros((1, 64), dtype=np.float32)
        bdm = np.zeros((512, 64), dtype=np.float32)
        wdd = np.empty((128, 512), dtype=np.float32)
        bdd = np.empty((1, 512), dtype=np.float32)
        xvd_cols = np.empty((512, B), dtype=np.float32)
        exl_cols = np.empty((512, B), dtype=np.float32)
        wampq = np.empty((128, 64), dtype=np.float32)
        biasq = np.empty((128, 64), dtype=np.float32)
        eba = np.empty((1, 64), dtype=np.float32)
        ebb = np.empty((1, 64), dtype=np.float32)

        for j, ch in enumerate(chs):
            w = 8 * ch
            nw = WIN_OF_J[j]
            arr = Wd5[:, ch, :, :ch, :].reshape(CDIM, FOUT, w)  # [k, q, w]
            arrp = np.zeros((CDIM, FOUT, 128 * nw), dtype=np.float32)
            arrp[:, :, :w] = arr
            # blocks in (q, a) order: [q, a, i, k]
            bl = arrp.reshape(CDIM, FOUT, nw, 128).transpose(1, 2, 3, 0)
            blocks.append(np.ascontiguousarray(bl.reshape(FOUT * nw, 128, CDIM)))
            bdj = bd4[ch, :, :ch, :].reshape(FOUT, w)  # [q, w]
            if w >= 128:
                a64 = arr.astype(np.float64)
                G = np.matmul(a64.transpose(1, 0, 2), a64.transpose(1, 2, 0))
                tr = np.trace(G, axis1=1, axis2=2)
                G += np.eye(CDIM)[None] * (1e-9 * tr[:, None, None] / CDIM)
                L = np.linalg.cholesky(G)  # [q, 128, 128], G = L @ L.T
                for q in range(FOUT):
                    lch[:, j * 8 + q, :] = L[q]
            else:
                for q in range(FOUT):
                    lch[:, j * 8 + q, :w] = arr[:, q, :]
            for q in range(FOUT):
                r = j * 8 + q
                crossq[:, r] = 2.0 * (arr[:, q, :] @ bdj[q])
                ebc[0, r] = bdj[q] @ bdj[q]
                bdm[:w, r] = bdj[q]
                wdd[:, r * 8 : (r + 1) * 8] = Wd5[:, ch, q, ch, :]
                bdd[0, r * 8 : (r + 1) * 8] = bd4[ch, q, ch, :]
                xvd_cols[r * 8 : (r + 1) * 8, :] = xvT[8 * ch : 8 * ch + 8, :]
                exl_cols[r * 8 : (r + 1) * 8, :] = exlT[8 * ch : 8 * ch + 8, :]
            rows = slice(ch * FOUT, (ch + 1) * FOUT)
            wampq[:, j * 8 : (j + 1) * 8] = Wa[:, rows]
            biasq[:, j * 8 : (j + 1) * 8] = Wb[:, rows]
            eba[0, j * 8 : (j + 1) * 8] = ba[rows]
            ebb[0, j * 8 : (j + 1) * 8] = bb[rows]

        wtk = np.concatenate(blocks, axis=0)  # [160, 128, 128] (blk, i, k)
        wtk = np.ascontiguousarray(
            wtk.transpose(1, 0, 2).reshape(128, NBLK * 128)
        )

        in_maps.append(
            {
                "ct": cT.astype(bf),
                "wtk": wtk.astype(bf),
                "lch": np.ascontiguousarray(lch.reshape(128, NLOC * 128)).astype(bf),
                "xvt": xvt,
                "wdd": wdd.astype(bf),
                "bdd": bdd.astype(bf),
                "xvdt": fold4(xvd_cols).astype(bf),
                "exlt": fold4(exl_cols).astype(bf),
                "sel16": sel16,
                "sel64": sel64,
                "wampq": wampq.astype(bf),
                "biasq": biasq.astype(bf),
                "eba": eba.astype(bf),
                "ebb": ebb.astype(bf),
                "crossq": crossq.astype(bf),
                "ebc": ebc.astype(bf),
                "bdm": fold4(bdm).astype(bf),
            }
        )
    return in_maps


def kernel(x, c, Wd, bd, Wa, ba, Wb, bb, _trace=False, _tmpdir=None):
    global _NC
    from concourse.bass_utils import run_bass_kernel_spmd

    if _NC is None:
        _NC = _build_nc()
    in_maps = _host_prep(x, c, Wd, bd, Wa, ba, Wb, bb)
    res = run_bass_kernel_spmd(
        _NC, in_maps, core_ids=list(range(NCORES)), trace=_trace, tmpdir=_tmpdir
    )

    out = np.empty((B, O, 2), dtype=np.float32)
    for k in range(NCORES):
        ok = res.results[k]["out"]
        for j, ch in enumerate(_channels(k)):
            out[:, ch * FOUT : (ch + 1) * FOUT, :] = ok[:, j * FOUT : (j + 1) * FOUT, :]
    if _trace:
        return out, res
    return out


# revision 19
# speedup vs baseline: 1.7221x; 1.0660x over previous
"""Trainium2 Bass kernel for nn_CLinear_6768868459230 (V-formulation).

Context-conditioned block-autoregressive linear layer (MAF-style):
  wdir = c @ Wd + bd                      [B, O, I]
  w    = exp(wdir)*mask_diag + wdir*mask_lower
  sqn  = sum(w^2, axis=I)
  y    = (w / sqrt(sqn) * exp(wamp)) @ xv + bias
  logdet = logsumexp over diag block of (wdir - 0.5 log sqn + wamp + xl)

Sharding: tensor-parallel over the O=512 output rows. Each of the 8 cores
owns 8 of the 64 channels, chosen as {k, 15-k, 16+k, 31-k, ...} so the
per-slot lower-width window count is identical on every core (shared
program).

Per row r the only heavy quantities are the two lower-block reductions
  B_r[b] = sum_i t[b,r,i]*xv[b,i]   and   A_r[b] = sum_i t[b,r,i]^2
with t = c @ W_r. Neither requires materializing t in batch-major layout:

  B_r = c . V_r           V_r = W_r @ xv   (PE, contract over i in
                          128-wide windows, accumulate in PSUM; [128k, 256b])
  A_r = ||L_r^T c||^2     L_r from a host-side Cholesky of the Gram
                          W_r W_r^T (compresses rows wider than 128), plus
                          host-folded cross/const terms for bd.

One DVE multiply (V*cT) and one ScalarE square (z^2) fill a combined
bf16 tile per row pair; a single 512-wide one-hot selector matmul per row
then reduces both quantities over the 128 partitions straight into the
PSUM accumulators (sqn|dot). All reductions run on the PE at 128-way
parallelism; the batch is one 256-wide free dim; PSUM start/stop follows
a one-start-per-bank discipline (2KB zero regions).

DMA rides all four DGE queues (sync/scalar/gpsimd/vector) ordered so the
PE never waits: a single combined warmup tensor gates the first matmuls,
weight streams are split to arrive just ahead of consumption.
"""

import numpy as np

NCH, FIN, FOUT, CDIM, B = 64, 8, 8, 128, 256
I = NCH * FIN
O = NCH * FOUT
NCORES = 8
NLOC = 64  # output rows per core
WIN_OF_J = [1, 1, 2, 2, 3, 3, 4, 4]  # 128-wide i-windows per slot
NBLK = 8 * sum(WIN_OF_J)  # 160 weight blocks per core

# warmup tensor column offsets (bf16, [128, 1536])
WM_CT, WM_WDD, WM_WAMP, WM_BIAS, WM_BDD, WM_EBA, WM_EBB, WM_END = (
    0, 256, 768, 832, 896, 1408, 1472, 1536)
# dx2 tensor column offsets (bf16, [128, 2688])
DX_XVD, DX_EXL, DX_BDM, DX_SEL16, DX_CROSS, DX_EBC, DX_END = (
    0, 1024, 2048, 2304, 2560, 2624, 2688)


def _channels(k):
    return [k, 15 - k, 16 + k, 31 - k, 32 + k, 47 - k, 48 + k, 63 - k]


_NC = None


def _build_nc():
    import concourse.bacc as bacc
    import concourse.tile as tile
    from concourse import mybir
    from concourse.masks import make_identity

    f32 = mybir.dt.float32
    bf16 = mybir.dt.bfloat16
    AF = mybir.ActivationFunctionType

    nc = bacc.Bacc(None, target_bir_lowering=False)

    d_warm = nc.dram_tensor("warm", [128, WM_END], bf16, kind="ExternalInput")
    d_xvt = nc.dram_tensor("xvt", [128, 4 * B], bf16, kind="ExternalInput")
    d_wtk = nc.dram_tensor("wtk", [128, NBLK * 128], bf16, kind="ExternalInput")
    d_lch = nc.dram_tensor("lch", [128, NLOC * 128], bf16, kind="ExternalInput")
    d_sel64 = nc.dram_tensor("sel64", [128, 64 * 64], bf16, kind="ExternalInput")
    d_dx2 = nc.dram_tensor("dx2", [128, DX_END], bf16, kind="ExternalInput")
    d_out = nc.dram_tensor("out", [B, NLOC, 2], f32, kind="ExternalOutput")

    blk_start = [8 * sum(WIN_OF_J[:j]) for j in range(8)]

    with tile.TileContext(nc) as tc:
        with (
            tc.tile_pool(name="consts", bufs=1) as consts,
            tc.tile_pool(name="scrP", bufs=8) as scrP,
            tc.tile_pool(name="scrD", bufs=2) as scrD,
            tc.tile_pool(name="asm", bufs=1) as asm,
            tc.tile_pool(name="pVp", bufs=2, space="PSUM") as pVp,
            tc.tile_pool(name="pZp", bufs=2, space="PSUM") as pZp,
            tc.tile_pool(name="paccp", bufs=1, space="PSUM") as paccp,
            tc.tile_pool(name="ptdp", bufs=1, space="PSUM") as ptdp,
            tc.tile_pool(name="pexp", bufs=1, space="PSUM") as pexp,
        ):
            # ---- DMAs across four DGE queues ----
            warm_sb = consts.tile([128, WM_END], bf16)
            nc.sync.dma_start(out=warm_sb, in_=d_warm[:, :])
            xvt_sb = consts.tile([128, 4, B], bf16)
            nc.scalar.dma_start(out=xvt_sb, in_=d_xvt[:, :])
            wtk_sb = consts.tile([128, NBLK, 128], bf16)
            for j in range(6):  # scalar queue: slots 0-5 (after xvt)
                b0, b1 = blk_start[j], blk_start[j] + 8 * WIN_OF_J[j]
                nc.scalar.dma_start(
                    out=wtk_sb[:, b0:b1, :], in_=d_wtk[:, b0 * 128 : b1 * 128]
                )
            lch_sb = consts.tile([128, NLOC, 128], bf16)
            for g in range(4):  # gpsimd queue: lch first
                nc.gpsimd.dma_start(
                    out=lch_sb[:, 16 * g : 16 * (g + 1), :],
                    in_=d_lch[:, 16 * g * 128 : 16 * (g + 1) * 128],
                )
            for j in (6, 7):  # then slots 6-7
                b0, b1 = blk_start[j], blk_start[j] + 8 * WIN_OF_J[j]
                nc.gpsimd.dma_start(
                    out=wtk_sb[:, b0:b1, :], in_=d_wtk[:, b0 * 128 : b1 * 128]
                )
            sel64_sb = consts.tile([128, 64, 64], bf16)
            nc.sync.dma_start(out=sel64_sb, in_=d_sel64[:, :])
            dx2_sb = consts.tile([128, DX_END], bf16)
            nc.sync.dma_start(out=dx2_sb, in_=d_dx2[:, :])

            ct_sb = warm_sb[:, WM_CT : WM_CT + 256]
            wdd_sb = warm_sb[:, WM_WDD : WM_WDD + 512]
            wampq_sb = warm_sb[:, WM_WAMP : WM_WAMP + 64]
            biasq_sb = warm_sb[:, WM_BIAS : WM_BIAS + 64]
            bdd_sb = warm_sb[0:1, WM_BDD : WM_BDD + 512]
            eba_sb = warm_sb[0:1, WM_EBA : WM_EBA + 64]
            ebb_sb = warm_sb[0:1, WM_EBB : WM_EBB + 64]

            ones1 = consts.tile([1, B], bf16)
            nc.vector.memset(ones1, 1.0)
            id64 = consts.tile([64, 64], f32)
            make_identity(nc, id64)

            ct_b2 = ct_sb.unsqueeze(1).broadcast_to([CDIM, 2, B])

            # ---- acc regions: bank0 = (sqn | dot), bank1 = (LDS | wamp) ----
            acc = paccp.tile([64, 4, 256], f32, name="acc", tag="acc")

            # warmup: wamp -> acc[:,3] (opens bank1); bias -> pex -> SBUF
            nc.tensor.matmul(
                acc[:, 3, :], wampq_sb, ct_sb, start=True, stop=False,
                skip_group_check=True,
            )
            nc.tensor.matmul(
                acc[:, 3, :], eba_sb, ones1, start=False, stop=False,
                skip_group_check=True,
            )
            pex = pexp.tile([128, 256], f32, name="pex", tag="pex")
            nc.tensor.matmul(
                pex[0:64, :], biasq_sb, ct_sb, start=True, stop=False,
                skip_group_check=True,
            )
            nc.tensor.matmul(
                pex[0:64, :], ebb_sb, ones1, start=False, stop=True,
                skip_group_check=True,
            )
            bias_sb = asm.tile([64, 256], f32, name="bias", tag="bias")
            nc.scalar.copy(bias_sb, pex[0:64, :])

            # diag produce + exps (PE warmup while wtk streams in)
            E_g, E2_g = [], []
            for g in range(2):
                ptd = ptdp.tile([128, 2, 256], f32, name="ptd", tag="ptd")
                for u in range(2):
                    a = 2 * g + u
                    nc.tensor.matmul(
                        ptd[:, u, :],
                        wdd_sb[:, a * 128 : (a + 1) * 128],
                        ct_sb,
                        start=(u == 0),
                        stop=False,
                        skip_group_check=True,
                    )
                    nc.tensor.matmul(
                        ptd[:, u, :],
                        bdd_sb[:, a * 128 : (a + 1) * 128],
                        ones1,
                        start=False,
                        stop=(u == 1),
                        skip_group_check=True,
                    )
                E = scrD.tile([128, 2, 256], bf16, name="E", tag="E")
                nc.scalar.activation(out=E, in_=ptd, func=AF.Exp)
                E2 = scrD.tile([128, 2, 256], bf16, name="E2", tag="E2")
                nc.scalar.activation(out=E2, in_=ptd, func=AF.Exp, scale=2.0)
                E_g.append(E)
                E2_g.append(E2)

            # diag products for dot/LDS (DVE); selector matmuls ride `late`
            Pd_g, Pl_g = [], []
            for g in range(2):
                xvd_w = dx2_sb[:, DX_XVD + 512 * g : DX_XVD + 512 * (g + 1)]
                exl_w = dx2_sb[:, DX_EXL + 512 * g : DX_EXL + 512 * (g + 1)]
                Pd = scrD.tile([128, 2, 256], bf16, name="Pd", tag="Pd")
                nc.vector.tensor_mul(
                    Pd, E_g[g], xvd_w.rearrange("p (u b) -> p u b", b=256)
                )
                Pl = scrD.tile([128, 2, 256], bf16, name="Pl", tag="Pl")
                nc.vector.tensor_mul(
                    Pl, E_g[g], exl_w.rearrange("p (u b) -> p u b", b=256)
                )
                Pd_g.append(Pd)
                Pl_g.append(Pl)

            crossq_sb = dx2_sb[:, DX_CROSS : DX_CROSS + 64]
            ebc_sb = dx2_sb[0:1, DX_EBC : DX_EBC + 64]
            late = []
            late.append(lambda: nc.tensor.matmul(
                acc[:, 0, :], crossq_sb, ct_sb, start=False, stop=False,
                skip_group_check=True))
            late.append(lambda: nc.tensor.matmul(
                acc[:, 0, :], ebc_sb, ones1, start=False, stop=False,
                skip_group_check=True))
            for a_ in range(4):
                late.append(lambda a=a_: nc.tensor.matmul(
                    acc[:, 1, :],
                    dx2_sb[:, DX_BDM + 64 * a : DX_BDM + 64 * (a + 1)],
                    xvt_sb[:, a, :],
                    start=False, stop=False, skip_group_check=True))
            for g in range(2):
                for u in range(2):
                    a_ = 2 * g + u
                    late.append(lambda g=g, u=u, a=a_: nc.tensor.matmul(
                        acc[:, 0, :],
                        dx2_sb[:, DX_SEL16 + 64 * a : DX_SEL16 + 64 * (a + 1)],
                        E2_g[g][:, u, :],
                        start=False, stop=False, skip_group_check=True))
                    late.append(lambda g=g, u=u, a=a_: nc.tensor.matmul(
                        acc[:, 1, :],
                        dx2_sb[:, DX_SEL16 + 64 * a : DX_SEL16 + 64 * (a + 1)],
                        Pd_g[g][:, u, :],
                        start=False, stop=False, skip_group_check=True))
                    late.append(lambda g=g, u=u, a=a_: nc.tensor.matmul(
                        acc[:, 2, :],
                        dx2_sb[:, DX_SEL16 + 64 * a : DX_SEL16 + 64 * (a + 1)],
                        Pl_g[g][:, u, :],
                        start=False, stop=(a == 3), skip_group_check=True))

            # ---- merged pair loop ----
            blk = 0
            pend = []
            for t in range(32):
                pV = pVp.tile([128, 2, 256], f32, name="pV", tag="pV")
                for q in range(2):
                    r = 2 * t + q
                    nw = WIN_OF_J[r // 8]
                    for a in range(nw):
                        nc.tensor.matmul(
                            pV[:, q, :],
                            wtk_sb[:, blk, :],
                            xvt_sb[:, a, :],
                            start=(q == 0 and a == 0),
                            stop=(q == 1 and a == nw - 1),
                            skip_group_check=True,
                        )
                        blk += 1
                pz = pZp.tile([128, 2, 256], f32, name="pz", tag="pz")
                for q in range(2):
                    r = 2 * t + q
                    nc.tensor.matmul(
                        pz[:, q, :], lch_sb[:, r, :], ct_sb,
                        start=(q == 0), stop=(q == 1), skip_group_check=True,
                    )
                comb = scrP.tile([128, 2, 2, B], bf16, name="comb", tag="comb")
                nc.vector.tensor_mul(comb[:, :, 1, :], pV, ct_b2)
                nc.scalar.activation(out=comb[:, :, 0, :], in_=pz, func=AF.Square)
                pend.append((t, comb))
                if len(pend) > 4:
                    tq, cq = pend.pop(0)
                    for q in range(2):
                        r = 2 * tq + q
                        nc.tensor.matmul(
                            acc[:, 0:2, :],
                            sel64_sb[:, r, :],
                            cq[:, q, :, :],
                            start=(r == 0),
                            stop=False,
                            skip_group_check=True,
                        )
                    if t >= 10 and late:
                        late.pop(0)()
                        if late:
                            late.pop(0)()
            for tq, cq in pend:
                for q in range(2):
                    r = 2 * tq + q
                    nc.tensor.matmul(
                        acc[:, 0:2, :],
                        sel64_sb[:, r, :],
                        cq[:, q, :, :],
                        start=(r == 0),
                        stop=(r == 63 and not late),
                        skip_group_check=True,
                    )
            for fn in late:
                fn()
            late = []

            # ---- assembly (all [64, 256]) ----
            l1 = asm.tile([64, 256], f32, name="l1", tag="l1")
            nc.scalar.activation(out=l1, in_=acc[:, 0, :], func=AF.Ln)
            l2 = asm.tile([64, 256], f32, name="l2", tag="l2")
            nc.scalar.activation(out=l2, in_=acc[:, 2, :], func=AF.Ln)
            mh = asm.tile([64, 256], f32, name="mh", tag="mh")
            nc.scalar.mul(mh, l1, -0.5)
            u_t = asm.tile([64, 256], f32, name="u", tag="u")
            nc.vector.tensor_add(u_t, acc[:, 3, :], mh)
            sc = asm.tile([64, 256], f32, name="sc", tag="sc")
            nc.scalar.activation(out=sc, in_=u_t, func=AF.Exp)
            yv = asm.tile([64, 256], f32, name="yv", tag="yv")
            nc.vector.tensor_mul(yv, acc[:, 1, :], sc)
            yb = asm.tile([64, 256], f32, name="yb", tag="yb")
            nc.vector.tensor_add(yb, yv, bias_sb)
            ld = asm.tile([64, 256], f32, name="ld", tag="ld")
            nc.vector.tensor_add(ld, u_t, l2)

            # ---- transpose to [B, NLOC] and write out ----
            pT = pexp.tile([128, 256], f32, name="pex", tag="pex")
            for i, (src, o0) in enumerate(
                ((yb, 0), (yb, 64), (ld, 128), (ld, 192))
            ):
                nc.tensor.matmul(
                    pT[:, o0 : o0 + 64],
                    src[:, (i % 2) * 128 : (i % 2) * 128 + 128],
                    id64,
                    is_transpose=True,
                    start=(i == 0),
                    stop=(i == 3),
                    skip_group_check=True,
                )
            for h in range(2):
                ob = asm.tile([128, 64, 2], f32, name=f"ob{h}", tag=f"ob{h}")
                nc.vector.tensor_copy(out=ob[:, :, 0], in_=pT[:, 64 * h : 64 * h + 64])
                nc.vector.tensor_copy(
                    out=ob[:, :, 1], in_=pT[:, 128 + 64 * h : 192 + 64 * h]
                )
                nc.sync.dma_start(out=d_out[128 * h : 128 * (h + 1), :, :], in_=ob)

    nc.compile()
    return nc


def _host_prep(x, c, Wd, bd, Wa, ba, Wb, bb):
    """Build the 8 per-core input maps."""
    import ml_dtypes

    bf = ml_dtypes.bfloat16
    x = np.ascontiguousarray(x, dtype=np.float32)
    c = np.ascontiguousarray(c, dtype=np.float32)
    Wd5 = np.ascontiguousarray(Wd, dtype=np.float32).reshape(CDIM, NCH, FOUT, NCH, FIN)
    bd4 = np.ascontiguousarray(bd, dtype=np.float32).reshape(NCH, FOUT, NCH, FIN)
    Wa = np.asarray(Wa, dtype=np.float32)
    Wb = np.asarray(Wb, dtype=np.float32)
    ba = np.asarray(ba, dtype=np.float32)
    bb = np.asarray(bb, dtype=np.float32)

    cT = np.ascontiguousarray(c.T)  # [128, 256]
    xv = x[:, :, 0]
    xl = x[:, :, 1]
    xvT = np.ascontiguousarray(xv.T)  # [512, 256]
    exlT = np.exp(xl).T  # [512, 256]

    def fold4(a512):  # [512, N] -> [128, 4*N] window-major per partition
        N = a512.shape[1]
        return np.ascontiguousarray(
            a512.reshape(4, 128, N).transpose(1, 0, 2).reshape(128, 4 * N)
        )

    xvt = fold4(xvT).astype(bf)

    sel16 = np.zeros((128, 4, 64), dtype=np.float32)
    p = np.arange(128)
    for a in range(4):
        sel16[p, a, 16 * a + p // 8] = 1.0
    sel64 = np.ascontiguousarray(
        np.broadcast_to(np.eye(64, dtype=np.float32), (128, 64, 64)).reshape(128, 4096)
    ).astype(bf)

    in_maps = []
    for k in range(NCORES):
        chs = _channels(k)
        blocks = []
        lch = np.zeros((128, NLOC, 128), dtype=np.float32)
        crossq = np.zeros((128, 64), dtype=np.float32)
        ebc = np.zeros((64,), dtype=np.float32)
        bdm = np.zeros((512, 64), dtype=np.float32)
        wdd = np.empty((128, 512), dtype=np.float32)
        bdd = np.zeros((512,), dtype=np.float32)
        xvd_cols = np.empty((512, B), dtype=np.float32)
        exl_cols = np.empty((512, B), dtype=np.float32)
        wampq = np.empty((128, 64), dtype=np.float32)
        biasq = np.empty((128, 64), dtype=np.float32)
        eba = np.zeros((64,), dtype=np.float32)
        ebb = np.zeros((64,), dtype=np.float32)

        for j, ch in enumerate(chs):
            w = 8 * ch
            nw = WIN_OF_J[j]
            arr = Wd5[:, ch, :, :ch, :].reshape(CDIM, FOUT, w)  # [k, q, w]
            arrp = np.zeros((CDIM, FOUT, 128 * nw), dtype=np.float32)
            arrp[:, :, :w] = arr
            bl = arrp.reshape(CDIM, FOUT, nw, 128).transpose(1, 2, 3, 0)
            blocks.append(np.ascontiguousarray(bl.reshape(FOUT * nw, 128, CDIM)))
            bdj = bd4[ch, :, :ch, :].reshape(FOUT, w)  # [q, w]
            if w >= 128:
                a64 = arr.astype(np.float64)
                G = np.matmul(a64.transpose(1, 0, 2), a64.transpose(1, 2, 0))
                tr = np.trace(G, axis1=1, axis2=2)
                G += np.eye(CDIM)[None] * (1e-9 * tr[:, None, None] / CDIM)
                L = np.linalg.cholesky(G)  # [q, 128, 128], G = L @ L.T
                for q in range(FOUT):
                    lch[:, j * 8 + q, :] = L[q]
            else:
                for q in range(FOUT):
                    lch[:, j * 8 + q, :w] = arr[:, q, :]
            for q in range(FOUT):
                r = j * 8 + q
                crossq[:, r] = 2.0 * (arr[:, q, :] @ bdj[q])
                ebc[r] = bdj[q] @ bdj[q]
                bdm[:w, r] = bdj[q]
                wdd[:, r * 8 : (r + 1) * 8] = Wd5[:, ch, q, ch, :]
                bdd[r * 8 : (r + 1) * 8] = bd4[ch, q, ch, :]
                xvd_cols[r * 8 : (r + 1) * 8, :] = xvT[8 * ch : 8 * ch + 8, :]
                exl_cols[r * 8 : (r + 1) * 8, :] = exlT[8 * ch : 8 * ch + 8, :]
            rows = slice(ch * FOUT, (ch + 1) * FOUT)
            wampq[:, j * 8 : (j + 1) * 8] = Wa[:, rows]
            biasq[:, j * 8 : (j + 1) * 8] = Wb[:, rows]
            eba[j * 8 : (j + 1) * 8] = ba[rows]
            ebb[j * 8 : (j + 1) * 8] = bb[rows]

        wtk = np.concatenate(blocks, axis=0)  # [160, 128, 128] (blk, i, k)
        wtk = np.ascontiguousarray(
            wtk.transpose(1, 0, 2).reshape(128, NBLK * 128)
        )

        warm = np.zeros((128, WM_END), dtype=np.float32)
        warm[:, WM_CT : WM_CT + 256] = cT
        warm[:, WM_WDD : WM_WDD + 512] = wdd
        warm[:, WM_WAMP : WM_WAMP + 64] = wampq
        warm[:, WM_BIAS : WM_BIAS + 64] = biasq
        warm[0, WM_BDD : WM_BDD + 512] = bdd
        warm[0, WM_EBA : WM_EBA + 64] = eba
        warm[0, WM_EBB : WM_EBB + 64] = ebb

        dx2 = np.zeros((128, DX_END), dtype=np.float32)
        dx2[:, DX_XVD : DX_XVD + 1024] = fold4(xvd_cols)
        dx2[:, DX_EXL : DX_EXL + 1024] = fold4(exl_cols)
        dx2[:, DX_BDM : DX_BDM + 256] = fold4(bdm)
        dx2[:, DX_SEL16 : DX_SEL16 + 256] = sel16.reshape(128, 256)
        dx2[:, DX_CROSS : DX_CROSS + 64] = crossq
        dx2[0, DX_EBC : DX_EBC + 64] = ebc

        in_maps.append(
            {
                "warm": warm.astype(bf),
                "xvt": xvt,
                "wtk": wtk.astype(bf),
                "lch": np.ascontiguousarray(lch.reshape(128, NLOC * 128)).astype(bf),
                "sel64": sel64,
                "dx2": dx2.astype(bf),
            }
        )
    return in_maps


def kernel(x, c, Wd, bd, Wa, ba, Wb, bb, _trace=False, _tmpdir=None):
    global _NC
    from concourse.bass_utils import run_bass_kernel_spmd

    if _NC is None:
        _NC = _build_nc()
    in_maps = _host_prep(x, c, Wd, bd, Wa, ba, Wb, bb)
    res = run_bass_kernel_spmd(
        _NC, in_maps, core_ids=list(range(NCORES)), trace=_trace, tmpdir=_tmpdir
    )

    out = np.empty((B, O, 2), dtype=np.float32)
    for k in range(NCORES):
        ok = res.results[k]["out"]
        for j, ch in enumerate(_channels(k)):
            out[:, ch * FOUT : (ch + 1) * FOUT, :] = ok[:, j * FOUT : (j + 1) * FOUT, :]
    if _trace:
        return out, res
    return out


# revision 20
# speedup vs baseline: 1.9041x; 1.1057x over previous
"""Trainium2 Bass kernel for nn_CLinear_6768868459230 (V-formulation).

Context-conditioned block-autoregressive linear layer (MAF-style):
  wdir = c @ Wd + bd                      [B, O, I]
  w    = exp(wdir)*mask_diag + wdir*mask_lower
  sqn  = sum(w^2, axis=I)
  y    = (w / sqrt(sqn) * exp(wamp)) @ xv + bias
  logdet = logsumexp over diag block of (wdir - 0.5 log sqn + wamp + xl)

Sharding: tensor-parallel over the O=512 output rows. Each of the 8 cores
owns 8 of the 64 channels, chosen as {k, 15-k, 16+k, 31-k, ...} so the
per-slot lower-width window count is identical on every core (shared
program).

Per row r the only heavy quantities are the two lower-block reductions
  B_r[b] = sum_i t[b,r,i]*xv[b,i]   and   A_r[b] = sum_i t[b,r,i]^2
with t = c @ W_r. Neither requires materializing t in batch-major layout:

  B_r = c . V_r           V_r = W_r @ xv   (PE, contract over i in
                          128-wide windows, accumulate in PSUM; [128k, 256b])
  A_r = ||L_r^T c||^2     L_r from a host-side Cholesky of the Gram
                          W_r W_r^T (compresses rows wider than 128), plus
                          host-folded cross/const terms for bd.

One DVE multiply (V*cT) and one ScalarE square (z^2) fill a combined
bf16 tile per row pair; a single 512-wide one-hot selector matmul per row
then reduces both quantities over the 128 partitions straight into the
PSUM accumulators (sqn|dot). All reductions run on the PE at 128-way
parallelism; the batch is one 256-wide free dim; PSUM start/stop follows
a one-start-per-bank discipline (2KB zero regions).

DMA rides all four DGE queues (sync/scalar/gpsimd/vector) ordered so the
PE never waits: a single combined warmup tensor gates the first matmuls,
weight streams are split to arrive just ahead of consumption.
"""

import numpy as np

NCH, FIN, FOUT, CDIM, B = 64, 8, 8, 128, 256
I = NCH * FIN
O = NCH * FOUT
NCORES = 8
NLOC = 64  # output rows per core
WIN_OF_J = [1, 1, 2, 2, 3, 3, 4, 4]  # 128-wide i-windows per slot
NBLK = 8 * sum(WIN_OF_J)  # 160 weight blocks per core

# warmup tensor column offsets (bf16, [128, 1536])
WM_CT, WM_WDD, WM_WAMP, WM_BIAS, WM_BDD, WM_EBA, WM_EBB, WM_END = (
    0, 256, 768, 832, 896, 1408, 1472, 1536)
# dx2 tensor column offsets (bf16, [128, 2688])
DX_XVD, DX_EXL, DX_BDM, DX_SEL16, DX_CROSS, DX_EBC, DX_END = (
    0, 1024, 2048, 2304, 2560, 2624, 2688)


def _channels(k):
    return [k, 15 - k, 16 + k, 31 - k, 32 + k, 47 - k, 48 + k, 63 - k]


_NC = None


def _build_nc():
    import concourse.bacc as bacc
    import concourse.tile as tile
    from concourse import mybir
    from concourse.masks import make_identity

    f32 = mybir.dt.float32
    bf16 = mybir.dt.bfloat16
    AF = mybir.ActivationFunctionType

    nc = bacc.Bacc(None, target_bir_lowering=False)

    d_warm = nc.dram_tensor("warm", [128, WM_END], bf16, kind="ExternalInput")
    d_xvt = nc.dram_tensor("xvt", [128, 4 * B], bf16, kind="ExternalInput")
    d_wtk = nc.dram_tensor("wtk", [128, NBLK * 128], bf16, kind="ExternalInput")
    d_lch = nc.dram_tensor("lch", [128, NLOC * 128], bf16, kind="ExternalInput")
    d_sel64 = nc.dram_tensor("sel64", [128, 64 * 64], bf16, kind="ExternalInput")
    d_dx2 = nc.dram_tensor("dx2", [128, DX_END], bf16, kind="ExternalInput")
    d_out = nc.dram_tensor("out", [B, NLOC, 2], f32, kind="ExternalOutput")

    blk_start = [8 * sum(WIN_OF_J[:j]) for j in range(8)]

    with tile.TileContext(nc) as tc:
        with (
            tc.tile_pool(name="consts", bufs=1) as consts,
            tc.tile_pool(name="scrP", bufs=8) as scrP,
            tc.tile_pool(name="scrD", bufs=2) as scrD,
            tc.tile_pool(name="asm", bufs=1) as asm,
            tc.tile_pool(name="pVp", bufs=2, space="PSUM") as pVp,
            tc.tile_pool(name="pZp", bufs=2, space="PSUM") as pZp,
            tc.tile_pool(name="paccp", bufs=1, space="PSUM") as paccp,
            tc.tile_pool(name="ptdp", bufs=1, space="PSUM") as ptdp,
            tc.tile_pool(name="pexp", bufs=1, space="PSUM") as pexp,
        ):
            # ---- DMAs across four DGE queues ----
            warm_sb = consts.tile([128, WM_END], bf16)
            nc.sync.dma_start(out=warm_sb, in_=d_warm[:, :])
            xvt_sb = consts.tile([128, 4, B], bf16)
            nc.sync.dma_start(out=xvt_sb, in_=d_xvt[:, :])
            wtk_sb = consts.tile([128, NBLK, 128], bf16)
            for j in range(6):  # scalar queue: slots 0-5
                b0, b1 = blk_start[j], blk_start[j] + 8 * WIN_OF_J[j]
                nc.scalar.dma_start(
                    out=wtk_sb[:, b0:b1, :], in_=d_wtk[:, b0 * 128 : b1 * 128]
                )
            lch_sb = consts.tile([128, NLOC, 128], bf16)
            for g in range(4):  # gpsimd queue: lch first
                nc.gpsimd.dma_start(
                    out=lch_sb[:, 16 * g : 16 * (g + 1), :],
                    in_=d_lch[:, 16 * g * 128 : 16 * (g + 1) * 128],
                )
            for j in (6, 7):  # then slots 6-7
                b0, b1 = blk_start[j], blk_start[j] + 8 * WIN_OF_J[j]
                nc.gpsimd.dma_start(
                    out=wtk_sb[:, b0:b1, :], in_=d_wtk[:, b0 * 128 : b1 * 128]
                )
            sel64_sb = consts.tile([128, 64, 64], bf16)
            nc.sync.dma_start(out=sel64_sb[:, 0:16, :], in_=d_sel64[:, 0 : 16 * 64])
            dx2_sb = consts.tile([128, DX_END], bf16)
            nc.sync.dma_start(out=dx2_sb, in_=d_dx2[:, :])
            nc.sync.dma_start(
                out=sel64_sb[:, 16:64, :], in_=d_sel64[:, 16 * 64 : 64 * 64]
            )

            ct_sb = warm_sb[:, WM_CT : WM_CT + 256]
            wdd_sb = warm_sb[:, WM_WDD : WM_WDD + 512]
            wampq_sb = warm_sb[:, WM_WAMP : WM_WAMP + 64]
            biasq_sb = warm_sb[:, WM_BIAS : WM_BIAS + 64]
            bdd_sb = warm_sb[0:1, WM_BDD : WM_BDD + 512]
            eba_sb = warm_sb[0:1, WM_EBA : WM_EBA + 64]
            ebb_sb = warm_sb[0:1, WM_EBB : WM_EBB + 64]

            ones1 = consts.tile([1, B], bf16)
            nc.vector.memset(ones1, 1.0)
            id64 = consts.tile([64, 64], f32)
            make_identity(nc, id64)

            ct_b2 = ct_sb.unsqueeze(1).broadcast_to([CDIM, 2, B])

            # ---- acc regions: bank0 = (sqn | dot), bank1 = (LDS | wamp) ----
            acc = paccp.tile([64, 4, 256], f32, name="acc", tag="acc")

            # warmup: wamp -> acc[:,3] (opens bank1); bias -> pex -> SBUF
            nc.tensor.matmul(
                acc[:, 3, :], wampq_sb, ct_sb, start=True, stop=False,
                skip_group_check=True,
            )
            nc.tensor.matmul(
                acc[:, 3, :], eba_sb, ones1, start=False, stop=False,
                skip_group_check=True,
            )
            pex = pexp.tile([128, 256], f32, name="pex", tag="pex")
            nc.tensor.matmul(
                pex[0:64, :], biasq_sb, ct_sb, start=True, stop=False,
                skip_group_check=True,
            )
            nc.tensor.matmul(
                pex[0:64, :], ebb_sb, ones1, start=False, stop=True,
                skip_group_check=True,
            )
            bias_sb = asm.tile([64, 256], f32, name="bias", tag="bias")
            nc.scalar.copy(bias_sb, pex[0:64, :])

            # diag produce + exps (PE warmup while wtk streams in)
            E_g, E2_g = [], []
            for g in range(2):
                ptd = ptdp.tile([128, 2, 256], f32, name="ptd", tag="ptd")
                for u in range(2):
                    a = 2 * g + u
                    nc.tensor.matmul(
                        ptd[:, u, :],
                        wdd_sb[:, a * 128 : (a + 1) * 128],
                        ct_sb,
                        start=(u == 0),
                        stop=False,
                        skip_group_check=True,
                    )
                    nc.tensor.matmul(
                        ptd[:, u, :],
                        bdd_sb[:, a * 128 : (a + 1) * 128],
                        ones1,
                        start=False,
                        stop=(u == 1),
                        skip_group_check=True,
                    )
                E = scrD.tile([128, 2, 256], bf16, name="E", tag="E")
                nc.scalar.activation(out=E, in_=ptd, func=AF.Exp)
                E2 = scrD.tile([128, 2, 256], bf16, name="E2", tag="E2")
                nc.scalar.activation(out=E2, in_=ptd, func=AF.Exp, scale=2.0)
                E_g.append(E)
                E2_g.append(E2)

            # diag products for dot/LDS (DVE) are emitted mid-loop (t==8,
            # once dx2 has landed) so they don't stall the comb TT stream;
            # their selector matmuls ride `late`.
            Pd_g, Pl_g = [], []

            def emit_diag_products():
                for g in range(2):
                    xvd_w = dx2_sb[:, DX_XVD + 512 * g : DX_XVD + 512 * (g + 1)]
                    exl_w = dx2_sb[:, DX_EXL + 512 * g : DX_EXL + 512 * (g + 1)]
                    Pd = scrD.tile([128, 2, 256], bf16, name="Pd", tag="Pd")
                    nc.vector.tensor_mul(
                        Pd, E_g[g], xvd_w.rearrange("p (u b) -> p u b", b=256)
                    )
                    Pl = scrD.tile([128, 2, 256], bf16, name="Pl", tag="Pl")
                    nc.vector.tensor_mul(
                        Pl, E_g[g], exl_w.rearrange("p (u b) -> p u b", b=256)
                    )
                    Pd_g.append(Pd)
                    Pl_g.append(Pl)

            crossq_sb = dx2_sb[:, DX_CROSS : DX_CROSS + 64]
            ebc_sb = dx2_sb[0:1, DX_EBC : DX_EBC + 64]
            late = []
            late.append(lambda: nc.tensor.matmul(
                acc[:, 0, :], crossq_sb, ct_sb, start=False, stop=False,
                skip_group_check=True))
            late.append(lambda: nc.tensor.matmul(
                acc[:, 0, :], ebc_sb, ones1, start=False, stop=False,
                skip_group_check=True))
            for a_ in range(4):
                late.append(lambda a=a_: nc.tensor.matmul(
                    acc[:, 1, :],
                    dx2_sb[:, DX_BDM + 64 * a : DX_BDM + 64 * (a + 1)],
                    xvt_sb[:, a, :],
                    start=False, stop=False, skip_group_check=True))
            for g in range(2):
                for u in range(2):
                    a_ = 2 * g + u
                    late.append(lambda g=g, u=u, a=a_: nc.tensor.matmul(
                        acc[:, 0, :],
                        dx2_sb[:, DX_SEL16 + 64 * a : DX_SEL16 + 64 * (a + 1)],
                        E2_g[g][:, u, :],
                        start=False, stop=False, skip_group_check=True))
                    late.append(lambda g=g, u=u, a=a_: nc.tensor.matmul(
                        acc[:, 1, :],
                        dx2_sb[:, DX_SEL16 + 64 * a : DX_SEL16 + 64 * (a + 1)],
                        Pd_g[g][:, u, :],
                        start=False, stop=False, skip_group_check=True))
                    late.append(lambda g=g, u=u, a=a_: nc.tensor.matmul(
                        acc[:, 2, :],
                        dx2_sb[:, DX_SEL16 + 64 * a : DX_SEL16 + 64 * (a + 1)],
                        Pl_g[g][:, u, :],
                        start=False, stop=(a == 3), skip_group_check=True))

            # ---- merged pair loop ----
            blk = 0
            pend = []
            for t in range(32):
                pV = pVp.tile([128, 2, 256], f32, name="pV", tag="pV")
                for q in range(2):
                    r = 2 * t + q
                    nw = WIN_OF_J[r // 8]
                    for a in range(nw):
                        nc.tensor.matmul(
                            pV[:, q, :],
                            wtk_sb[:, blk, :],
                            xvt_sb[:, a, :],
                            start=(q == 0 and a == 0),
                            stop=(q == 1 and a == nw - 1),
                            skip_group_check=True,
                        )
                        blk += 1
                pz = pZp.tile([128, 2, 256], f32, name="pz", tag="pz")
                for q in range(2):
                    r = 2 * t + q
                    nc.tensor.matmul(
                        pz[:, q, :], lch_sb[:, r, :], ct_sb,
                        start=(q == 0), stop=(q == 1), skip_group_check=True,
                    )
                comb = scrP.tile([128, 2, 2, B], bf16, name="comb", tag="comb")
                nc.vector.tensor_mul(comb[:, :, 1, :], pV, ct_b2)
                nc.scalar.activation(out=comb[:, :, 0, :], in_=pz, func=AF.Square)
                if t == 8:
                    emit_diag_products()
                pend.append((t, comb))
                if len(pend) > 4:
                    tq, cq = pend.pop(0)
                    for q in range(2):
                        r = 2 * tq + q
                        nc.tensor.matmul(
                            acc[:, 0:2, :],
                            sel64_sb[:, r, :],
                            cq[:, q, :, :],
                            start=(r == 0),
                            stop=False,
                            skip_group_check=True,
                        )
                    if t >= 12 and late:
                        late.pop(0)()
                        if late:
                            late.pop(0)()
            for tq, cq in pend:
                for q in range(2):
                    r = 2 * tq + q
                    nc.tensor.matmul(
                        acc[:, 0:2, :],
                        sel64_sb[:, r, :],
                        cq[:, q, :, :],
                        start=(r == 0),
                        stop=(r == 63 and not late),
                        skip_group_check=True,
                    )
            for fn in late:
                fn()
            late = []

            # ---- assembly (all [64, 256]); ld finishes first so its
            # transposes overlap the yb chain ----
            l1 = asm.tile([64, 256], f32, name="l1", tag="l1")
            nc.scalar.activation(out=l1, in_=acc[:, 0, :], func=AF.Ln)
            l2 = asm.tile([64, 256], f32, name="l2", tag="l2")
            nc.scalar.activation(out=l2, in_=acc[:, 2, :], func=AF.Ln)
            mh = asm.tile([64, 256], f32, name="mh", tag="mh")
            nc.vector.tensor_scalar_mul(mh, l1, -0.5)
            u_t = asm.tile([64, 256], f32, name="u", tag="u")
            nc.vector.tensor_add(u_t, acc[:, 3, :], mh)
            ld = asm.tile([64, 256], f32, name="ld", tag="ld")
            nc.vector.tensor_add(ld, u_t, l2)
            sc = asm.tile([64, 256], f32, name="sc", tag="sc")
            nc.scalar.activation(out=sc, in_=u_t, func=AF.Exp)
            pT = pexp.tile([128, 256], f32, name="pex", tag="pex")
            for i in range(2):
                nc.tensor.matmul(
                    pT[:, 128 + 64 * i : 192 + 64 * i],
                    ld[:, i * 128 : (i + 1) * 128],
                    id64,
                    is_transpose=True,
                    start=(i == 0),
                    stop=False,
                    skip_group_check=True,
                )
            yv = asm.tile([64, 256], f32, name="yv", tag="yv")
            nc.vector.tensor_mul(yv, acc[:, 1, :], sc)
            yb = asm.tile([64, 256], f32, name="yb", tag="yb")
            nc.vector.tensor_add(yb, yv, bias_sb)
            for i in range(2):
                nc.tensor.matmul(
                    pT[:, 64 * i : 64 * i + 64],
                    yb[:, i * 128 : (i + 1) * 128],
                    id64,
                    is_transpose=True,
                    start=False,
                    stop=(i == 1),
                    skip_group_check=True,
                )
            for h in range(2):
                ob = asm.tile([128, 64, 2], f32, name=f"ob{h}", tag=f"ob{h}")
                nc.vector.tensor_copy(out=ob[:, :, 0], in_=pT[:, 64 * h : 64 * h + 64])
                nc.vector.tensor_copy(
                    out=ob[:, :, 1], in_=pT[:, 128 + 64 * h : 192 + 64 * h]
                )
                nc.sync.dma_start(out=d_out[128 * h : 128 * (h + 1), :, :], in_=ob)

    nc.compile()
    return nc


def _host_prep(x, c, Wd, bd, Wa, ba, Wb, bb):
    """Build the 8 per-core input maps."""
    import ml_dtypes

    bf = ml_dtypes.bfloat16
    x = np.ascontiguousarray(x, dtype=np.float32)
    c = np.ascontiguousarray(c, dtype=np.float32)
    Wd5 = np.ascontiguousarray(Wd, dtype=np.float32).reshape(CDIM, NCH, FOUT, NCH, FIN)
    bd4 = np.ascontiguousarray(bd, dtype=np.float32).reshape(NCH, FOUT, NCH, FIN)
    Wa = np.asarray(Wa, dtype=np.float32)
    Wb = np.asarray(Wb, dtype=np.float32)
    ba = np.asarray(ba, dtype=np.float32)
    bb = np.asarray(bb, dtype=np.float32)

    cT = np.ascontiguousarray(c.T)  # [128, 256]
    xv = x[:, :, 0]
    xl = x[:, :, 1]
    xvT = np.ascontiguousarray(xv.T)  # [512, 256]
    exlT = np.exp(xl).T  # [512, 256]

    def fold4(a512):  # [512, N] -> [128, 4*N] window-major per partition
        N = a512.shape[1]
        return np.ascontiguousarray(
            a512.reshape(4, 128, N).transpose(1, 0, 2).reshape(128, 4 * N)
        )

    xvt = fold4(xvT).astype(bf)

    sel16 = np.zeros((128, 4, 64), dtype=np.float32)
    p = np.arange(128)
    for a in range(4):
        sel16[p, a, 16 * a + p // 8] = 1.0
    sel64 = np.ascontiguousarray(
        np.broadcast_to(np.eye(64, dtype=np.float32), (128, 64, 64)).reshape(128, 4096)
    ).astype(bf)

    in_maps = []
    for k in range(NCORES):
        chs = _channels(k)
        blocks = []
        lch = np.zeros((128, NLOC, 128), dtype=np.float32)
        crossq = np.zeros((128, 64), dtype=np.float32)
        ebc = np.zeros((64,), dtype=np.float32)
        bdm = np.zeros((512, 64), dtype=np.float32)
        wdd = np.empty((128, 512), dtype=np.float32)
        bdd = np.zeros((512,), dtype=np.float32)
        xvd_cols = np.empty((512, B), dtype=np.float32)
        exl_cols = np.empty((512, B), dtype=np.float32)
        wampq = np.empty((128, 64), dtype=np.float32)
        biasq = np.empty((128, 64), dtype=np.float32)
        eba = np.zeros((64,), dtype=np.float32)
        ebb = np.zeros((64,), dtype=np.float32)

        for j, ch in enumerate(chs):
            w = 8 * ch
            nw = WIN_OF_J[j]
            arr = Wd5[:, ch, :, :ch, :].reshape(CDIM, FOUT, w)  # [k, q, w]
            arrp = np.zeros((CDIM, FOUT, 128 * nw), dtype=np.float32)
            arrp[:, :, :w] = arr
            bl = arrp.reshape(CDIM, FOUT, nw, 128).transpose(1, 2, 3, 0)
            blocks.append(np.ascontiguousarray(bl.reshape(FOUT * nw, 128, CDIM)))
            bdj = bd4[ch, :, :ch, :].reshape(FOUT, w)  # [q, w]
            if w >= 128:
                a64 = arr.astype(np.float64)
                G = np.matmul(a64.transpose(1, 0, 2), a64.transpose(1, 2, 0))
                tr = np.trace(G, axis1=1, axis2=2)
                G += np.eye(CDIM)[None] * (1e-9 * tr[:, None, None] / CDIM)
                L = np.linalg.cholesky(G)  # [q, 128, 128], G = L @ L.T
                for q in range(FOUT):
                    lch[:, j * 8 + q, :] = L[q]
            else:
                for q in range(FOUT):
                    lch[:, j * 8 + q, :w] = arr[:, q, :]
            for q in range(FOUT):
                r = j * 8 + q
                crossq[:, r] = 2.0 * (arr[:, q, :] @ bdj[q])
                ebc[r] = bdj[q] @ bdj[q]
                bdm[:w, r] = bdj[q]
                wdd[:, r * 8 : (r + 1) * 8] = Wd5[:, ch, q, ch, :]
                bdd[r * 8 : (r + 1) * 8] = bd4[ch, q, ch, :]
                xvd_cols[r * 8 : (r + 1) * 8, :] = xvT[8 * ch : 8 * ch + 8, :]
                exl_cols[r * 8 : (r + 1) * 8, :] = exlT[8 * ch : 8 * ch + 8, :]
            rows = slice(ch * FOUT, (ch + 1) * FOUT)
            wampq[:, j * 8 : (j + 1) * 8] = Wa[:, rows]
            biasq[:, j * 8 : (j + 1) * 8] = Wb[:, rows]
            eba[j * 8 : (j + 1) * 8] = ba[rows]
            ebb[j * 8 : (j + 1) * 8] = bb[rows]

        wtk = np.concatenate(blocks, axis=0)  # [160, 128, 128] (blk, i, k)
        wtk = np.ascontiguousarray(
            wtk.transpose(1, 0, 2).reshape(128, NBLK * 128)
        )

        warm = np.zeros((128, WM_END), dtype=np.float32)
        warm[:, WM_CT : WM_CT + 256] = cT
        warm[:, WM_WDD : WM_WDD + 512] = wdd
        warm[:, WM_WAMP : WM_WAMP + 64] = wampq
        warm[:, WM_BIAS : WM_BIAS + 64] = biasq
        warm[0, WM_BDD : WM_BDD + 512] = bdd
        warm[0, WM_EBA : WM_EBA + 64] = eba
        warm[0, WM_EBB : WM_EBB + 64] = ebb

        dx2 = np.zeros((128, DX_END), dtype=np.float32)
        dx2[:, DX_XVD : DX_XVD + 1024] = fold4(xvd_cols)
        dx2[:, DX_EXL : DX_EXL + 1024] = fold4(exl_cols)
        dx2[:, DX_BDM : DX_BDM + 256] = fold4(bdm)
        dx2[:, DX_SEL16 : DX_SEL16 + 256] = sel16.reshape(128, 256)
        dx2[:, DX_CROSS : DX_CROSS + 64] = crossq
        dx2[0, DX_EBC : DX_EBC + 64] = ebc

        in_maps.append(
            {
                "warm": warm.astype(bf),
                "xvt": xvt,
                "wtk": wtk.astype(bf),
                "lch": np.ascontiguousarray(lch.reshape(128, NLOC * 128)).astype(bf),
                "sel64": sel64,
                "dx2": dx2.astype(bf),
            }
        )
    return in_maps


def kernel(x, c, Wd, bd, Wa, ba, Wb, bb, _trace=False, _tmpdir=None):
    global _NC
    from concourse.bass_utils import run_bass_kernel_spmd

    if _NC is None:
        _NC = _build_nc()
    in_maps = _host_prep(x, c, Wd, bd, Wa, ba, Wb, bb)
    res = run_bass_kernel_spmd(
        _NC, in_maps, core_ids=list(range(NCORES)), trace=_trace, tmpdir=_tmpdir
    )

    out = np.empty((B, O, 2), dtype=np.float32)
    for k in range(NCORES):
        ok = res.results[k]["out"]
        for j, ch in enumerate(_channels(k)):
            out[:, ch * FOUT : (ch + 1) * FOUT, :] = ok[:, j * FOUT : (j + 1) * FOUT, :]
    if _trace:
        return out, res
    return out
